# revision 1
# baseline (speedup 1.0000x reference)
"""GCN (6-layer GCNConv) Trainium2 Bass kernel — v2.

Data-parallel over batch (1 mesh per NeuronCore). Per layer
out = A_hat @ (x @ W) + b with A_hat = D^-1/2 (A+I) D^-1/2 shared across batch
and layers.

v2 structure (per core):
  - Host: symmetric-norm edge list WITHOUT self-loops (their contribution is
    added on-device as a PE transpose of the diag-scaled h tile, accumulated
    into the same PSUM segment-sum group). Nodes are relabeled (degree-balanced
    bin packing) so every 128-node dst tile has <= C*128 in-edges; edges are
    grouped per dst tile and padded to C chunks of 128.
  - Device: phases interleave scatter(i) with dense(i+1) per dst tile. The
    scatter's feature-major output tile (stage, SBUF) is consumed directly as
    the next dense matmul's lhsT — activations never round-trip through DRAM
    between layers; only the node-major h gather tables do.
  - Layer 1 uses the rank-1 structure of the broadcast image features:
    h1 = verts @ W1[:3] + (img @ W1[3:]) broadcast over nodes (host-computed).
  - Layer 5 scatter runs in orientation A (node-major out, self-loops kept as
    real edges) to produce the gather table for layer 6's message-first pass.
  - Layer 6: message passing first (64-wide), then the 64->3 matmul.
"""
import sys
import time

sys.path.insert(0, "/opt/trn_rl_repo")
import numpy as np
from contextlib import ExitStack

import concourse.bass as bass
import concourse.mybir as mybir
import concourse.tile as tile
from concourse.bass_utils import run_bass_kernel_spmd
from concourse.masks import make_identity

P = 128
F32 = mybir.dt.float32
I32 = mybir.dt.int32

_msw_ctr = [0]


def _split_multiwaits(nc, max_waits=1):
    """This walrus build rejects >1 sync wait per instruction: split extras
    onto preceding same-engine NOPs."""
    for f in nc.m.functions:
        for b in f.blocks:
            out, changed = [], False
            for inst in b.instructions:
                si = getattr(inst, "sync_info", None)
                waits = list(si.on_wait) if si is not None else []
                if len(waits) > max_waits:
                    changed = True
                    for w in waits[:-max_waits]:
                        _msw_ctr[0] += 1
                        nop = mybir.InstNoOp(name=f"msw-{_msw_ctr[0]}", ins=[], outs=[])
                        nop.engine = inst.engine
                        nop.sync_info = mybir.SyncInfo(on_wait=[w], on_update=[])
                        out.append(nop)
                    si.on_wait = waits[-max_waits:]
                out.append(inst)
            if changed:
                b.instructions = out
    return nc


def _pack_graph(src, dst, N):
    """Relabel nodes into degree-balanced 128-node tiles (no self-loops in the
    edge list). Returns device arrays [128, T*C] plus the with-self-loops
    variant [128, T*(C+1)] used by layer 5."""
    T = (N + P - 1) // P
    NP = T * P
    indeg = np.bincount(dst, minlength=N)          # no-loop in-degree
    C = max(1, int(np.ceil(len(src) / (T * P))))

    order = np.argsort(-indeg, kind="stable")
    while True:
        cap = C * P
        load = np.zeros(T, np.int64)
        count = np.zeros(T, np.int64)
        assign = np.empty(N, np.int64)
        ok = True
        for v in order:
            d = int(indeg[v])
            best_t, best_rem = -1, -1
            for t in range(T):
                if count[t] < P:
                    rem = cap - load[t]
                    if rem > best_rem:
                        best_rem, best_t = rem, t
            if best_t < 0 or load[best_t] + d > cap:
                ok = False
                break
            assign[v] = best_t
            load[best_t] += d
            count[best_t] += 1
        if ok:
            break
        C += 1

    perm = np.full(NP, -1, np.int64)
    new_of_old = np.empty(N, np.int64)
    cursor = np.zeros(T, np.int64)
    for v in range(N):
        t = assign[v]
        nid = t * P + cursor[t]
        cursor[t] += 1
        perm[nid] = v
        new_of_old[v] = nid

    # symmetric normalization (degree INCLUDES self-loops, per GCN)
    deg = (indeg + 1).astype(np.float32)
    dinv = (1.0 / np.sqrt(deg, dtype=np.float32)).astype(np.float32)
    norm = (dinv[src] * dinv[dst]).astype(np.float32)

    src_n = new_of_old[src]
    dst_n = new_of_old[dst]
    tile_of_e = dst_n // P
    order_e = np.argsort(tile_of_e, kind="stable")
    src_n, dst_n, norm = src_n[order_e], dst_n[order_e], norm[order_e]
    tile_of_e = tile_of_e[order_e]

    gsrc = np.zeros((T, C, P), np.int32)
    slot = np.zeros((T, C, P), np.float32)
    nrm = np.zeros((T, C, P), np.float32)
    starts = np.searchsorted(tile_of_e, np.arange(T + 1))
    for t in range(T):
        lo, hi = starts[t], starts[t + 1]
        n_e = hi - lo
        assert n_e <= C * P, (t, n_e, C * P)
        fs = np.zeros(C * P, np.int32)
        fl = np.zeros(C * P, np.float32)
        fn = np.zeros(C * P, np.float32)
        fs[:n_e] = src_n[lo:hi]
        fl[:n_e] = (dst_n[lo:hi] - t * P).astype(np.float32)
        fn[:n_e] = norm[lo:hi]
        gsrc[t] = fs.reshape(C, P)
        slot[t] = fl.reshape(C, P)
        nrm[t] = fn.reshape(C, P)

    # per-(slot, tile) dinv^2 for the on-device self-loop term (0 for dummies)
    dinv_new = np.zeros(NP, np.float32)
    valid = perm >= 0
    dinv_new[valid] = dinv[perm[valid]]
    dinv2 = (dinv_new ** 2).reshape(T, P).T.copy()   # [128, T]

    def dev(a):
        return np.ascontiguousarray(a.transpose(2, 0, 1).reshape(P, -1))

    return dict(NP=NP, T=T, C=C, perm=perm, dinv2=np.ascontiguousarray(dinv2),
                gsrc=dev(gsrc), slot=dev(slot), norm=dev(nrm))


def _build_nc(NP, T, C, FM, F5, FO):
    import os
    scratch = int(os.environ.get("KBASS_SCRATCH", "16384"))
    MD = mybir.dt.bfloat16 if os.environ.get("KBASS_MSGDT", "f32") == "bf16" else F32
    nc = bass.Bass(dynamic_dma_scratch_size=scratch)
    TC = T * C
    C5 = C + 1
    KM = FM // P
    K5 = FM // P

    d = {}
    d["xT1"] = nc.dram_tensor("xT1", [3, NP], F32, kind="ExternalInput")
    d["hcrep"] = nc.dram_tensor("hcrep", [P, FM], F32, kind="ExternalInput")
    d["W1v"] = nc.dram_tensor("W1v", [3, FM], F32, kind="ExternalInput")
    for i in (2, 3, 4):
        d[f"W{i}"] = nc.dram_tensor(f"W{i}", [FM, FM], F32, kind="ExternalInput")
    d["W5"] = nc.dram_tensor("W5", [FM, F5], F32, kind="ExternalInput")
    d["W6"] = nc.dram_tensor("W6", [F5, FO], F32, kind="ExternalInput")
    d["B14"] = nc.dram_tensor("B14", [P, 4 * KM], F32, kind="ExternalInput")
    d["b5rep"] = nc.dram_tensor("b5rep", [P, F5], F32, kind="ExternalInput")
    d["b6rep"] = nc.dram_tensor("b6rep", [P, FO], F32, kind="ExternalInput")
    d["gsrc"] = nc.dram_tensor("gsrc", [P, TC], I32, kind="ExternalInput")
    d["slot"] = nc.dram_tensor("slot", [P, TC], F32, kind="ExternalInput")
    d["normv"] = nc.dram_tensor("normv", [P, TC], F32, kind="ExternalInput")
    d["B14R"] = nc.dram_tensor("B14R", [P, 4 * FM], F32, kind="ExternalInput")
    d["dinv2"] = nc.dram_tensor("dinv2", [P, T], F32, kind="ExternalInput")
    out_d = nc.dram_tensor("out", [NP, FO], F32, kind="ExternalOutput")

    h512 = [nc.dram_tensor(f"h{i}", [NP, FM], MD, kind="Internal") for i in (1, 2, 3, 4)]
    h5_d = nc.dram_tensor("h5", [NP, F5], MD, kind="Internal")
    x6_d = nc.dram_tensor("x6", [NP, F5], MD, kind="Internal")

    Ident = mybir.ActivationFunctionType.Identity
    Relu = mybir.ActivationFunctionType.Relu

    with tile.TileContext(nc) as tc:
        with ExitStack() as ctx:
            res = ctx.enter_context(tc.tile_pool(name="res", bufs=1))
            gsrc_sb = res.tile([P, TC], I32)
            slot_sb = res.tile([P, TC], F32)
            norm_sb = res.tile([P, TC], F32)
            for name, t_sb in [("gsrc", gsrc_sb), ("slot", slot_sb), ("normv", norm_sb)]:
                nc.sync.dma_start(out=t_sb[:], in_=d[name][:, :])
            B14R_sb = res.tile([P, 4 * FM], F32)
            nc.sync.dma_start(out=B14R_sb[:], in_=d["B14R"][:, :])
            iota_i = res.tile([P, P], I32)
            nc.gpsimd.iota(iota_i[:], pattern=[[1, P]], base=0, channel_multiplier=0)
            iota_f = res.tile([P, P], F32)
            nc.vector.tensor_copy(out=iota_f[:], in_=iota_i[:])
            ident_sb = res.tile([P, P], F32)
            make_identity(nc, ident_sb[:])
            hcrep_sb = res.tile([P, FM], F32)
            nc.sync.dma_start(out=hcrep_sb[:], in_=d["hcrep"][:, :])
            B14_sb = res.tile([P, 4 * KM], F32)
            nc.sync.dma_start(out=B14_sb[:], in_=d["B14"][:, :])
            b5rep_sb = res.tile([P, F5], F32)
            nc.sync.dma_start(out=b5rep_sb[:], in_=d["b5rep"][:, :])
            b6rep_sb = res.tile([P, FO], F32)
            nc.sync.dma_start(out=b6rep_sb[:], in_=d["b6rep"][:, :])
            dinv2_sb = res.tile([P, T], F32)
            nc.sync.dma_start(out=dinv2_sb[:], in_=d["dinv2"][:, :])
            if MD is F32:
                iota_m, slot_m, norm_m = iota_f, slot_sb, norm_sb
            else:
                iota_m = res.tile([P, P], MD)
                nc.vector.tensor_copy(out=iota_m[:], in_=iota_f[:])
                slot_m = res.tile([P, TC], MD)
                nc.vector.tensor_copy(out=slot_m[:], in_=slot_sb[:])
                norm_m = res.tile([P, TC], MD)
                nc.vector.tensor_copy(out=norm_m[:], in_=norm_sb[:])


            # ---- layer 1 dense ----
            with tc.tile_pool(name="l1", bufs=1) as l1p, \
                 tc.tile_pool(name="l1ps", bufs=2, space="PSUM") as l1ps, \
                 tc.tile_pool(name="l1sb", bufs=3) as l1sb:
                xT1_sb = l1p.tile([3, NP], F32)
                nc.sync.dma_start(out=xT1_sb[:], in_=d["xT1"][:, :])
                W1v_sb = l1p.tile([3, FM], F32)
                nc.sync.dma_start(out=W1v_sb[:], in_=d["W1v"][:, :])
                for n in range(T):
                    ph = l1ps.tile([P, FM], F32, tag="ph")
                    nc.tensor.matmul(out=ph[:], lhsT=xT1_sb[:, n * P:(n + 1) * P],
                                     rhs=W1v_sb[:], start=True, stop=True)
                    hs = l1sb.tile([P, FM], MD, tag="hs")
                    nc.vector.tensor_add(out=hs[:], in0=ph[:], in1=hcrep_sb[:])
                    nc.sync.dma_start(out=h512[0][n * P:(n + 1) * P, :], in_=hs[:])

            def build_onehot(sp, t, c_cnt, slot_src, norm_src, tag):
                oh = sp.tile([P, c_cnt * P], MD, tag=tag, name=f"oh_{tag}_{t}")
                oh3 = oh[:].rearrange("p (c j) -> p c j", c=c_cnt)
                nc.vector.tensor_tensor(
                    out=oh3,
                    in0=slot_src[:, t * c_cnt:(t + 1) * c_cnt]
                        .rearrange("p (c u) -> p c u", u=1).to_broadcast([P, c_cnt, P]),
                    in1=iota_f[:].rearrange("p (u j) -> p u j", u=1)
                        .to_broadcast([P, c_cnt, P]),
                    op=mybir.AluOpType.is_equal,
                )
                nc.vector.tensor_tensor(
                    out=oh3, in0=oh3,
                    in1=norm_src[:, t * c_cnt:(t + 1) * c_cnt]
                        .rearrange("p (c u) -> p c u", u=1).to_broadcast([P, c_cnt, P]),
                    op=mybir.AluOpType.mult,
                )
                return oh

            # ---- merged phases: scatter(i) + dense(i+1), i = 1..4 ----
            # layer i scatter consumes h512[i-1]; dense(i+1) writes h512[i] or h5
            for i in (1, 2, 3, 4):
                relu = i in (2, 4)
                h_src = h512[i - 1]
                F_out = FM if i < 4 else F5
                h_dst = h512[i] if i < 4 else h5_d
                W_d = d[f"W{i + 1}"]
                with tc.tile_pool(name=f"ph{i}", bufs=int(__import__("os").environ.get("KBASS_BUFS", "2"))) as sp, \
                     tc.tile_pool(name=f"ph{i}w", bufs=1) as wp, \
                     tc.tile_pool(name=f"ph{i}ps", bufs=2, space="PSUM") as pp, \
                     tc.tile_pool(name=f"ph{i}pt", bufs=2, space="PSUM") as pt, \
                     tc.tile_pool(name=f"ph{i}pd", bufs=2, space="PSUM") as pd:
                    W_sb = [wp.tile([P, F_out], F32, tag=f"w{k}", name=f"w{i}_{k}")
                            for k in range(KM)]
                    for k in range(KM):
                        nc.sync.dma_start(out=W_sb[k][:], in_=W_d[k * P:(k + 1) * P, :])
                    for t in range(T):
                        # self-loop + bias term: diag-scaled h tile + replicated b_i
                        hre = sp.tile([P, FM], MD, tag="hre", name=f"hre{i}_{t}")
                        nc.sync.dma_start(out=hre[:], in_=h_src[t * P:(t + 1) * P, :])
                        sfb = sp.tile([P, FM], F32, tag="sfb", name=f"sfb{i}_{t}")
                        nc.vector.tensor_scalar_mul(
                            out=sfb[:], in0=hre[:], scalar1=dinv2_sb[:, t:t + 1])
                        nc.vector.tensor_add(
                            out=sfb[:], in0=sfb[:],
                            in1=B14R_sb[:, (i - 1) * FM:i * FM])
                        msgs = []
                        for c in range(C):
                            mc = sp.tile([P, FM], MD, tag=f"msg{c}", name=f"msg{i}_{t}_{c}")
                            nc.gpsimd.indirect_dma_start(
                                out=mc[:],
                                out_offset=None,
                                in_=h_src[:, :],
                                in_offset=bass.IndirectOffsetOnAxis(
                                    ap=gsrc_sb[:, t * C + c:t * C + c + 1], axis=0),
                            )
                            msgs.append(mc)
                        oh = build_onehot(sp, t, C, slot_m, norm_m, "oh")
                        # orientation A: node-major segment sum, onehot stationary
                        pa = pp.tile([P, FM], F32, tag="pa", name=f"pa{i}_{t}")
                        for c in range(C):
                            nc.tensor.matmul(
                                out=pa[:], lhsT=oh[:, c * P:(c + 1) * P],
                                rhs=msgs[c][:],
                                start=(c == 0), stop=(c == C - 1))
                        node = sp.tile([P, FM], F32, tag="node", name=f"nd{i}_{t}")
                        nc.vector.tensor_add(out=node[:], in0=pa[:], in1=sfb[:])
                        if relu:
                            nc.vector.tensor_scalar_max(out=node[:], in0=node[:],
                                                        scalar1=0.0)
                        # to feature-major via PE transposes
                        ptr = pt.tile([P, FM], F32, tag="ptr", name=f"pt{i}_{t}")
                        stage = sp.tile([P, FM], F32, tag="stage", name=f"st{i}_{t}")
                        for fo in range(KM):
                            nc.tensor.matmul(
                                out=ptr[:, fo * P:(fo + 1) * P],
                                lhsT=node[:, fo * P:(fo + 1) * P],
                                rhs=ident_sb[:], is_transpose=True,
                                start=True, stop=True)
                            nc.scalar.activation(
                                out=stage[:, fo * P:(fo + 1) * P],
                                in_=ptr[:, fo * P:(fo + 1) * P],
                                func=Ident, bias=0.0)
                        # dense(i+1) for this tile, straight from stage
                        ph = pd.tile([P, F_out], F32, tag="ph", name=f"pd{i}_{t}")
                        for k in range(KM):
                            nc.tensor.matmul(out=ph[:], lhsT=stage[:, k * P:(k + 1) * P],
                                             rhs=W_sb[k][:], start=(k == 0),
                                             stop=(k == KM - 1))
                        hs = sp.tile([P, F_out], MD, tag="hs", name=f"hs{i}_{t}")
                        nc.vector.tensor_copy(out=hs[:], in_=ph[:])
                        nc.sync.dma_start(out=h_dst[t * P:(t + 1) * P, :], in_=hs[:])

            # ---- layer 5 scatter (orientation A, self-loop via DVE add) ----
            with tc.tile_pool(name="s5", bufs=2) as sp5, \
                 tc.tile_pool(name="s5ps", bufs=2, space="PSUM") as pp5:
                for t in range(T):
                    hre5 = sp5.tile([P, F5], MD, tag="hre5", name=f"hr5_{t}")
                    nc.sync.dma_start(out=hre5[:], in_=h5_d[t * P:(t + 1) * P, :])
                    sfb5 = sp5.tile([P, F5], F32, tag="sfb5", name=f"sf5_{t}")
                    nc.vector.tensor_scalar_mul(
                        out=sfb5[:], in0=hre5[:], scalar1=dinv2_sb[:, t:t + 1])
                    nc.vector.tensor_add(out=sfb5[:], in0=sfb5[:], in1=b5rep_sb[:])
                    msg = sp5.tile([P, C * F5], MD, tag="msg5", name=f"m5_{t}")
                    for c in range(C):
                        nc.gpsimd.indirect_dma_start(
                            out=msg[:, c * F5:(c + 1) * F5],
                            out_offset=None,
                            in_=h5_d[:, :],
                            in_offset=bass.IndirectOffsetOnAxis(
                                ap=gsrc_sb[:, t * C + c:t * C + c + 1], axis=0),
                        )
                    oh = build_onehot(sp5, t, C, slot_m, norm_m, "oh5")
                    pa = pp5.tile([P, F5], F32, tag="pa", name=f"pa_{t}")
                    for c in range(C):
                        nc.tensor.matmul(out=pa[:], lhsT=oh[:, c * P:(c + 1) * P],
                                         rhs=msg[:, c * F5:(c + 1) * F5],
                                         start=(c == 0), stop=(c == C - 1))
                    xo = sp5.tile([P, F5], MD, tag="xo5", name=f"xo_{t}")
                    nc.vector.tensor_add(out=xo[:], in0=pa[:], in1=sfb5[:])
                    nc.sync.dma_start(out=x6_d[t * P:(t + 1) * P, :], in_=xo[:])

            # ---- layer 6: scatter (orientation B) + dense, interleaved ----
            with tc.tile_pool(name="s6", bufs=2) as sp6, \
                 tc.tile_pool(name="s6w", bufs=1) as wp6, \
                 tc.tile_pool(name="s6ps", bufs=2, space="PSUM") as pp6, \
                 tc.tile_pool(name="s6pd", bufs=2, space="PSUM") as pd6:
                W6_sb = wp6.tile([F5, FO], F32)
                nc.sync.dma_start(out=W6_sb[:], in_=d["W6"][:, :])
                for t in range(T):
                    hre = sp6.tile([P, F5], MD, tag="hre6", name=f"hre6_{t}")
                    nc.sync.dma_start(out=hre[:], in_=x6_d[t * P:(t + 1) * P, :])
                    hsc = sp6.tile([P, F5], F32, tag="hsc6", name=f"hsc6_{t}")
                    nc.vector.tensor_scalar_mul(
                        out=hsc[:], in0=hre[:], scalar1=dinv2_sb[:, t:t + 1])
                    msg = sp6.tile([P, C * F5], MD, tag="msg6", name=f"m6_{t}")
                    for c in range(C):
                        nc.gpsimd.indirect_dma_start(
                            out=msg[:, c * F5:(c + 1) * F5],
                            out_offset=None,
                            in_=x6_d[:, :],
                            in_offset=bass.IndirectOffsetOnAxis(
                                ap=gsrc_sb[:, t * C + c:t * C + c + 1], axis=0),
                        )
                    oh = build_onehot(sp6, t, C, slot_m, norm_m, "oh6")
                    pg = pp6.tile([F5, P], F32, tag="pg", name=f"pg_{t}")
                    nc.tensor.matmul(out=pg[:], lhsT=hsc[:], rhs=ident_sb[:],
                                     is_transpose=True, start=True, stop=False,
                                     skip_group_check=True)
                    for c in range(C):
                        nc.tensor.matmul(out=pg[:], lhsT=msg[:, c * F5:(c + 1) * F5],
                                         rhs=oh[:, c * P:(c + 1) * P],
                                         start=False, stop=(c == C - 1),
                                         skip_group_check=True)
                    gst = sp6.tile([F5, P], F32, tag="gst", name=f"g_{t}")
                    nc.scalar.activation(out=gst[:], in_=pg[:], func=Ident, bias=0.0)
                    pf = pd6.tile([P, FO], F32, tag="pf", name=f"pf_{t}")
                    nc.tensor.matmul(out=pf[:], lhsT=gst[:], rhs=W6_sb[:],
                                     start=True, stop=True)
                    os_ = sp6.tile([P, FO], F32, tag="os", name=f"o_{t}")
                    nc.vector.tensor_add(out=os_[:], in0=pf[:], in1=b6rep_sb[:])
                    nc.sync.dma_start(out=out_d[t * P:(t + 1) * P, :], in_=os_[:])

    _split_multiwaits(nc)
    return nc


def _prepare(batch_vertices, img_features, edge_indices,
             W1, b1, W2, b2, W3, b3, W4, b4, W5, b5, W6, b6):
    B, N, _ = batch_vertices.shape
    FM = W1.shape[1]
    F5 = W5.shape[1]
    FO = W6.shape[1]

    ei = np.asarray(edge_indices).astype(np.int64)
    g = _pack_graph(ei[0], ei[1], N)
    NP, T, C, perm = g["NP"], g["T"], g["C"], g["perm"]

    KM = FM // P
    hc = img_features.astype(np.float32) @ W1[3:].astype(np.float32)

    valid = perm >= 0
    vperm = np.zeros((B, NP, 3), np.float32)
    vperm[:, valid, :] = batch_vertices[:, perm[valid], :]

    common = {
        "W1v": np.ascontiguousarray(W1[:3].astype(np.float32)),
        "W2": np.ascontiguousarray(W2.astype(np.float32)),
        "W3": np.ascontiguousarray(W3.astype(np.float32)),
        "W4": np.ascontiguousarray(W4.astype(np.float32)),
        "W5": np.ascontiguousarray(W5.astype(np.float32)),
        "W6": np.ascontiguousarray(W6.astype(np.float32)),
        "B14": np.ascontiguousarray(
            np.stack([b.reshape(KM, P).T for b in (b1, b2, b3, b4)],
                     axis=1).reshape(P, 4 * KM).astype(np.float32)),
        "b5rep": np.tile(b5.astype(np.float32), (P, 1)),
        "b6rep": np.tile(b6.astype(np.float32), (P, 1)),
        "gsrc": g["gsrc"], "slot": g["slot"], "normv": g["norm"],
        "B14R": np.ascontiguousarray(
            np.tile(np.concatenate([b1, b2, b3, b4]).astype(np.float32), (P, 1))),
        "dinv2": g["dinv2"],
    }
    in_maps = []
    for b in range(B):
        m = dict(common)
        m["xT1"] = np.ascontiguousarray(vperm[b].T)
        m["hcrep"] = np.tile(hc[b], (P, 1))
        in_maps.append(m)
    meta = dict(NP=NP, T=T, C=C, perm=perm, valid=valid, B=B, N=N,
                FM=FM, F5=F5, FO=FO)
    return in_maps, meta


_BUILD_CACHE = {}


def run(inputs, trace=False):
    in_maps, meta = _prepare(**inputs)
    key = (meta["NP"], meta["C"], meta["FM"], meta["F5"], meta["FO"])
    if key not in _BUILD_CACHE:
        t0 = time.time()
        _BUILD_CACHE[key] = _build_nc(meta["NP"], meta["T"], meta["C"],
                                      meta["FM"], meta["F5"], meta["FO"])
        print(f"[kernel] built bass program in {time.time()-t0:.1f}s", file=sys.stderr)
    nc = _BUILD_CACHE[key]
    B = meta["B"]
    res = run_bass_kernel_spmd(nc, in_maps, core_ids=list(range(B)), trace=trace)
    perm, valid, N = meta["perm"], meta["valid"], meta["N"]
    out = np.empty((B, N, meta["FO"]), np.float32)
    for b in range(B):
        dev = res.results[b]["out"]
        out[b, perm[valid], :] = dev[valid, :]
    return out, res


def kernel(**inputs) -> np.ndarray:
    out, _ = run(inputs)
    return out



# revision 15
# speedup vs baseline: 1.3353x; 1.3353x over previous
"""GCN (6-layer GCNConv) Trainium2 Bass kernel — v3.

Data-parallel over batch (1 mesh per NeuronCore). Per layer
out = A_hat @ (x @ W) + b with A_hat = D^-1/2 (A+I) D^-1/2 shared across
batch and layers.

v3 structure (vs v2):
  - bf16 datapath: all matmul operands bf16 (PE 1 cyc/row vs 4 for f32),
    f32 PSUM accumulation; gather tables bf16 (half the HBM traffic).
  - One 768-index dma_gather per dst tile replaces 6 indirect DMAs
    (~6x less Pool-engine SWDGE descriptor-generation time).
  - h tables are stored pre-scaled by dinv^2, so the self-loop term is a
    single identity matmul from the SBUF-resident copy of the tile (no
    DRAM re-read, no DVE scaling); the per-edge norm becomes
    dinv[dst]/dinv[src].
  - Bias (+ReLU) folded into the feature-major PSUM->SBUF activation
    copies as per-partition bias APs; layer-1 image-feature term as a
    rank-1 matmul. Steady-state DVE work is only the one-hot builds.
  - Layers 5/6 (64-wide) keep their tables padded to 128 columns so the
    gather rows stay 256B-aligned.
"""
import sys
import time

sys.path.insert(0, "/opt/trn_rl_repo")
import numpy as np
import ml_dtypes
from contextlib import ExitStack

import concourse.bass as bass
import concourse.bacc as bacc
import concourse.mybir as mybir
import concourse.tile as tile
from concourse.bass_utils import run_bass_kernel_spmd
from concourse.masks import make_identity

P = 128
F32 = mybir.dt.float32
BF16 = mybir.dt.bfloat16
I16 = mybir.dt.int16
BF = ml_dtypes.bfloat16

_msw_ctr = [0]


def _split_multiwaits(nc, max_waits=1):
    """This walrus build rejects >1 sync wait per instruction: split extras
    onto preceding same-engine NOPs."""
    for f in nc.m.functions:
        for b in f.blocks:
            out, changed = [], False
            for inst in b.instructions:
                si = getattr(inst, "sync_info", None)
                waits = list(si.on_wait) if si is not None else []
                if len(waits) > max_waits:
                    changed = True
                    for w in waits[:-max_waits]:
                        _msw_ctr[0] += 1
                        nop = mybir.InstNoOp(name=f"msw-{_msw_ctr[0]}", ins=[], outs=[])
                        nop.engine = inst.engine
                        nop.sync_info = mybir.SyncInfo(on_wait=[w], on_update=[])
                        out.append(nop)
                    si.on_wait = waits[-max_waits:]
                out.append(inst)
            if changed:
                b.instructions = out
    return nc


def _pack_graph(src, dst, N):
    """Relabel nodes into degree-balanced 128-node tiles (no self-loops in
    the edge list). Edge normalization is expressed relative to tables that
    store h*dinv^2: message weight = dinv[dst]/dinv[src]."""
    T = (N + P - 1) // P
    NP = T * P
    indeg = np.bincount(dst, minlength=N)          # no-loop in-degree
    C = max(1, int(np.ceil(len(src) / (T * P))))

    order = np.argsort(-indeg, kind="stable")
    while True:
        cap = C * P
        load = np.zeros(T, np.int64)
        count = np.zeros(T, np.int64)
        assign = np.empty(N, np.int64)
        ok = True
        for v in order:
            d = int(indeg[v])
            best_t, best_rem = -1, -1
            for t in range(T):
                if count[t] < P:
                    rem = cap - load[t]
                    if rem > best_rem:
                        best_rem, best_t = rem, t
            if best_t < 0 or load[best_t] + d > cap:
                ok = False
                break
            assign[v] = best_t
            load[best_t] += d
            count[best_t] += 1
        if ok:
            break
        C += 1

    perm = np.full(NP, -1, np.int64)
    new_of_old = np.empty(N, np.int64)
    cursor = np.zeros(T, np.int64)
    for v in range(N):
        t = assign[v]
        nid = t * P + cursor[t]
        cursor[t] += 1
        perm[nid] = v
        new_of_old[v] = nid

    # symmetric normalization (degree INCLUDES self-loops, per GCN)
    deg = (indeg + 1).astype(np.float32)
    dinv = (1.0 / np.sqrt(deg, dtype=np.float32)).astype(np.float32)
    # gathered message rows already carry dinv[src]^2
    norm = (dinv[dst] / dinv[src]).astype(np.float32)

    src_n = new_of_old[src]
    dst_n = new_of_old[dst]
    tile_of_e = dst_n // P
    order_e = np.argsort(tile_of_e, kind="stable")
    src_n, dst_n, norm = src_n[order_e], dst_n[order_e], norm[order_e]
    tile_of_e = tile_of_e[order_e]

    gsrc = np.zeros((T, C, P), np.int32)
    slot = np.zeros((T, C, P), np.float32)
    nrm = np.zeros((T, C, P), np.float32)
    starts = np.searchsorted(tile_of_e, np.arange(T + 1))
    for t in range(T):
        lo, hi = starts[t], starts[t + 1]
        n_e = hi - lo
        assert n_e <= C * P, (t, n_e, C * P)
        fs = np.zeros(C * P, np.int32)
        fl = np.zeros(C * P, np.float32)
        fn = np.zeros(C * P, np.float32)
        fs[:n_e] = src_n[lo:hi]
        fl[:n_e] = (dst_n[lo:hi] - t * P).astype(np.float32)
        fn[:n_e] = norm[lo:hi]
        gsrc[t] = fs.reshape(C, P)
        slot[t] = fl.reshape(C, P)
        nrm[t] = fn.reshape(C, P)

    # int16 index table for dma_gather: flat slot i = c*128 + j of tile t
    # lives at [i % 16, t*SW + i//16]
    SW = C * P // 16
    idx16 = np.zeros((P, T * SW), np.int16)
    for t in range(T):
        flat = gsrc[t].reshape(C * P).astype(np.int16)   # i = c*128 + j
        # wrapped [i%16, i//16], replicated to every 16-partition stripe
        # (each GPSIMD cpu reads indices from its own stripe)
        idx16[:, t * SW:(t + 1) * SW] = np.tile(flat.reshape(SW, 16).T, (8, 1))

    # per-(slot, tile) dinv^2 used to pre-scale h tables (0 for dummies)
    dinv_new = np.zeros(NP, np.float32)
    valid = perm >= 0
    dinv_new[valid] = dinv[perm[valid]]
    dinv2 = (dinv_new ** 2).reshape(T, P).T.copy()   # [128, T]

    def dev(a):
        return np.ascontiguousarray(a.transpose(2, 0, 1).reshape(P, -1))

    return dict(NP=NP, T=T, C=C, SW=SW, perm=perm,
                dinv2=np.ascontiguousarray(dinv2), idx16=idx16,
                slot=dev(slot).astype(BF), norm=dev(nrm).astype(BF))


def _build_nc(NP, T, C, SW, FM, F5, FO, split_mw=False):
    import os
    scratch = int(os.environ.get("KBASS_SCRATCH", "32768"))
    # Bacc: compile() auto-inserts GPSIMD library loads (dma_gather needs the
    # 'mlp' ucode) and lowers pseudo-instructions to valid ISA.
    nc = bacc.Bacc("TRN2", dynamic_dma_scratch_size=scratch)
    TC = T * C
    KM = FM // P
    NI = C * P   # gather indices per dst tile

    d = {}
    d["xT1"] = nc.dram_tensor("xT1", [3, NP], BF16, kind="ExternalInput")
    d["hcrow"] = nc.dram_tensor("hcrow", [1, FM], BF16, kind="ExternalInput")
    d["W1v"] = nc.dram_tensor("W1v", [3, FM], BF16, kind="ExternalInput")
    for i in (2, 3, 4):
        d[f"W{i}"] = nc.dram_tensor(f"W{i}", [FM, FM], BF16, kind="ExternalInput")
    d["W5"] = nc.dram_tensor("W5", [FM, F5], BF16, kind="ExternalInput")
    d["W6"] = nc.dram_tensor("W6", [F5, FO], BF16, kind="ExternalInput")
    d["B14"] = nc.dram_tensor("B14", [P, 4 * KM], F32, kind="ExternalInput")
    d["b5col"] = nc.dram_tensor("b5col", [F5, 1], F32, kind="ExternalInput")
    d["b6rep"] = nc.dram_tensor("b6rep", [P, FO], F32, kind="ExternalInput")
    d["ones1"] = nc.dram_tensor("ones1", [1, P], BF16, kind="ExternalInput")
    d["idx16"] = nc.dram_tensor("idx16", [P, T * SW], I16, kind="ExternalInput")
    d["slotb"] = nc.dram_tensor("slotb", [P, TC], BF16, kind="ExternalInput")
    d["normb"] = nc.dram_tensor("normb", [P, TC], BF16, kind="ExternalInput")
    d["dinv2"] = nc.dram_tensor("dinv2", [P, T], F32, kind="ExternalInput")
    out_d = nc.dram_tensor("out", [NP, FO], F32, kind="ExternalOutput")

    h512 = [nc.dram_tensor(f"h{i}", [NP, FM], BF16, kind="Internal")
            for i in (1, 2, 3, 4)]
    h5t = nc.dram_tensor("h5t", [NP, P], BF16, kind="Internal")
    x6t = nc.dram_tensor("x6t", [NP, P], BF16, kind="Internal")

    Ident = mybir.ActivationFunctionType.Identity
    Relu = mybir.ActivationFunctionType.Relu

    with tile.TileContext(nc) as tc:
        with ExitStack() as ctx:
            res = ctx.enter_context(tc.tile_pool(name="res", bufs=1))
            idx_sb = res.tile([P, T * SW], I16)
            slot_sb = res.tile([P, TC], BF16)
            norm_sb = res.tile([P, TC], BF16)
            B14_sb = res.tile([P, 4 * KM], F32)
            dinv2_sb = res.tile([P, T], F32)
            b5c_sb = res.tile([F5, 1], F32)
            b6r_sb = res.tile([P, FO], F32)
            ones_sb = res.tile([1, P], BF16)
            hcrow_sb = res.tile([1, FM], BF16)
            for name, t_sb in [("idx16", idx_sb), ("slotb", slot_sb),
                               ("normb", norm_sb), ("B14", B14_sb),
                               ("dinv2", dinv2_sb), ("b5col", b5c_sb),
                               ("b6rep", b6r_sb), ("ones1", ones_sb),
                               ("hcrow", hcrow_sb)]:
                nc.sync.dma_start(out=t_sb[:], in_=d[name][:, :])
            reg_ni = nc.gpsimd.to_reg(NI)   # shared gather index-count register
            iota_i = res.tile([P, P], mybir.dt.int32)
            nc.gpsimd.iota(iota_i[:], pattern=[[1, P]], base=0, channel_multiplier=0)
            iota_b = res.tile([P, P], BF16)
            nc.vector.tensor_copy(out=iota_b[:], in_=iota_i[:])
            ident_b = res.tile([P, P], BF16)
            make_identity(nc, ident_b[:])
            hres = [res.tile([P, FM], BF16, name=f"hres_{t}") for t in range(T)]
            # hres5 tiles are padded to 128 cols so the 64-wide tables keep
            # 256B gather rows; pad stays zero and is never computed on.
            hres5 = [res.tile([P, P], BF16, name=f"hres5_{t}") for t in range(T)]
            for t in range(T):
                nc.vector.memset(hres5[t][:, F5:P], 0.0)

            def build_onehot(sp, t, dt_, tag):
                oh = sp.tile([P, NI], dt_, tag=tag, name=f"oh_{tag}_{t}")
                oh3 = oh[:].rearrange("p (c j) -> p c j", c=C)
                nc.vector.tensor_tensor(
                    out=oh3,
                    in0=slot_sb[:, t * C:(t + 1) * C]
                        .rearrange("p (c u) -> p c u", u=1).to_broadcast([P, C, P]),
                    in1=iota_b[:].rearrange("p (u j) -> p u j", u=1)
                        .to_broadcast([P, C, P]),
                    op=mybir.AluOpType.is_equal,
                )
                nc.vector.tensor_tensor(
                    out=oh3, in0=oh3,
                    in1=norm_sb[:, t * C:(t + 1) * C]
                        .rearrange("p (c u) -> p c u", u=1).to_broadcast([P, C, P]),
                    op=mybir.AluOpType.mult,
                )
                return oh

            # ---- layer 1 dense: h1 = (verts @ W1[:3] + img@W1[3:]) * dinv2 ----
            with tc.tile_pool(name="l1", bufs=1) as l1p, \
                 tc.tile_pool(name="l1ps", bufs=2, space="PSUM") as l1ps:
                xT1_sb = l1p.tile([3, NP], BF16)
                nc.sync.dma_start(out=xT1_sb[:], in_=d["xT1"][:, :])
                W1v_sb = l1p.tile([3, FM], BF16)
                nc.sync.dma_start(out=W1v_sb[:], in_=d["W1v"][:, :])
                for t in range(T):
                    pd1 = l1ps.tile([P, FM], F32, tag="pd1", name=f"pd1_{t}")
                    nc.tensor.matmul(out=pd1[:], lhsT=xT1_sb[:, t * P:(t + 1) * P],
                                     rhs=W1v_sb[:], start=True, stop=False)
                    nc.tensor.matmul(out=pd1[:], lhsT=ones_sb[:], rhs=hcrow_sb[:],
                                     start=False, stop=True)
                    nc.scalar.activation(out=hres[t][:], in_=pd1[:], func=Ident,
                                         scale=dinv2_sb[:, t:t + 1])
                    nc.sync.dma_start(out=h512[0][t * P:(t + 1) * P, :],
                                      in_=hres[t][:])

            # ---- merged phases: scatter(i) + dense(i+1), i = 1..4 ----
            import os as _os
            SBUFS = int(_os.environ.get("KBASS_BUFS", "3"))
            for i in (1, 2, 3, 4):
                relu = i in (2, 4)
                h_src = h512[i - 1]
                F_out = FM if i < 4 else F5
                W_d = d[f"W{i + 1}"]
                with tc.tile_pool(name=f"ph{i}", bufs=SBUFS) as sp, \
                     tc.tile_pool(name=f"ph{i}w", bufs=1) as wp, \
                     tc.tile_pool(name=f"ph{i}ps", bufs=2, space="PSUM") as pp, \
                     tc.tile_pool(name=f"ph{i}pt", bufs=2, space="PSUM") as pt, \
                     tc.tile_pool(name=f"ph{i}pd", bufs=2, space="PSUM") as pd:
                    W_sb = [wp.tile([P, F_out], BF16, tag=f"w{k}", name=f"w{i}_{k}")
                            for k in range(KM)]
                    for k in range(KM):
                        nc.sync.dma_start(out=W_sb[k][:], in_=W_d[k * P:(k + 1) * P, :])
                    for t in range(T):
                        g = sp.tile([P, C * FM], BF16, tag="g", name=f"g{i}_{t}")
                        nc.gpsimd.dma_gather(
                            out_ap=g[:].rearrange("p (c f) -> p c f", c=C),
                            in_ap=h_src[:, :],
                            idxs_ap=idx_sb[:, t * SW:(t + 1) * SW],
                            num_idxs=NI, num_idxs_reg=reg_ni, elem_size=FM)
                        oh = build_onehot(sp, t, BF16, "oh")
                        pa = pp.tile([P, FM], F32, tag="pa", name=f"pa{i}_{t}")
                        # self-loop: h table rows are pre-scaled by dinv^2
                        nc.tensor.matmul(out=pa[:], lhsT=ident_b[:], rhs=hres[t][:],
                                         start=True, stop=False)
                        for c in range(C):
                            nc.tensor.matmul(
                                out=pa[:], lhsT=oh[:, c * P:(c + 1) * P],
                                rhs=g[:, c * FM:(c + 1) * FM],
                                start=False, stop=(c == C - 1))
                        node = sp.tile([P, FM], BF16, tag="node", name=f"nd{i}_{t}")
                        nc.scalar.activation(out=node[:], in_=pa[:], func=Ident)
                        ptr = pt.tile([P, FM], F32, tag="ptr", name=f"pt{i}_{t}")
                        stage = sp.tile([P, FM], BF16, tag="stage", name=f"st{i}_{t}")
                        for fo in range(KM):
                            # feature-major transpose as node_chunk^T @ I
                            nc.tensor.matmul(
                                out=ptr[:, fo * P:(fo + 1) * P],
                                lhsT=node[:, fo * P:(fo + 1) * P],
                                rhs=ident_b[:],
                                start=True, stop=True)
                            nc.scalar.activation(
                                out=stage[:, fo * P:(fo + 1) * P],
                                in_=ptr[:, fo * P:(fo + 1) * P],
                                func=Relu if relu else Ident,
                                bias=B14_sb[:, (i - 1) * KM + fo:(i - 1) * KM + fo + 1])
                        pdn = pd.tile([P, F_out], F32, tag="pdn", name=f"pd{i}_{t}")
                        for k in range(KM):
                            nc.tensor.matmul(out=pdn[:],
                                             lhsT=stage[:, k * P:(k + 1) * P],
                                             rhs=W_sb[k][:], start=(k == 0),
                                             stop=(k == KM - 1))
                        if i < 4:
                            nc.scalar.activation(out=hres[t][:], in_=pdn[:],
                                                 func=Ident,
                                                 scale=dinv2_sb[:, t:t + 1])
                            nc.sync.dma_start(out=h512[i][t * P:(t + 1) * P, :],
                                              in_=hres[t][:])
                        else:
                            nc.scalar.activation(out=hres5[t][:, 0:F5],
                                                 in_=pdn[:], func=Ident,
                                                 scale=dinv2_sb[:, t:t + 1])
                            nc.sync.dma_start(out=h5t[t * P:(t + 1) * P, :],
                                              in_=hres5[t][:])

            # ---- phase 5: x6 = (A_hat h5 + b5) * dinv2, 64-wide ----
            with tc.tile_pool(name="s5", bufs=SBUFS) as sp5, \
                 tc.tile_pool(name="s5ps", bufs=2, space="PSUM") as pp5, \
                 tc.tile_pool(name="s5pt", bufs=2, space="PSUM") as pt5:
                for t in range(T):
                    g5 = sp5.tile([P, C * P], BF16, tag="g5", name=f"g5_{t}")
                    nc.gpsimd.dma_gather(
                        out_ap=g5[:].rearrange("p (c f) -> p c f", c=C),
                        in_ap=h5t[:, :],
                        idxs_ap=idx_sb[:, t * SW:(t + 1) * SW],
                        num_idxs=NI, num_idxs_reg=reg_ni, elem_size=P)
                    oh5 = build_onehot(sp5, t, BF16, "oh5")
                    pg5 = pp5.tile([F5, P], F32, tag="pg5", name=f"pg5_{t}")
                    nc.tensor.matmul(out=pg5[:], lhsT=hres5[t][:, 0:F5],
                                     rhs=ident_b[:],
                                     start=True, stop=False,
                                     skip_group_check=True)
                    for c in range(C):
                        nc.tensor.matmul(out=pg5[:],
                                         lhsT=g5[:, c * P:c * P + F5],
                                         rhs=oh5[:, c * P:(c + 1) * P],
                                         start=False, stop=(c == C - 1),
                                         skip_group_check=True)
                    st6 = sp5.tile([F5, P], BF16, tag="st6", name=f"st6_{t}")
                    nc.scalar.activation(out=st6[:], in_=pg5[:], func=Ident,
                                         bias=b5c_sb[:, 0:1])
                    pt6 = pt5.tile([P, F5], F32, tag="pt6", name=f"pt6_{t}")
                    nc.tensor.matmul(out=pt6[:], lhsT=st6[:],
                                     rhs=ident_b[0:F5, 0:F5],
                                     start=True, stop=True)
                    nc.scalar.activation(out=hres5[t][:, 0:F5], in_=pt6[:],
                                         func=Ident,
                                         scale=dinv2_sb[:, t:t + 1])
                    nc.sync.dma_start(out=x6t[t * P:(t + 1) * P, :],
                                      in_=hres5[t][:])

            # ---- phase 6: out = (A_hat x6) @ W6 + b6 ----
            with tc.tile_pool(name="s6", bufs=SBUFS) as sp6, \
                 tc.tile_pool(name="s6w", bufs=1) as wp6, \
                 tc.tile_pool(name="s6ps", bufs=2, space="PSUM") as pp6, \
                 tc.tile_pool(name="s6pd", bufs=2, space="PSUM") as po6:
                W6_sb = wp6.tile([F5, FO], BF16)
                nc.sync.dma_start(out=W6_sb[:], in_=d["W6"][:, :])
                for t in range(T):
                    g6 = sp6.tile([P, C * P], BF16, tag="g6", name=f"g6_{t}")
                    nc.gpsimd.dma_gather(
                        out_ap=g6[:].rearrange("p (c f) -> p c f", c=C),
                        in_ap=x6t[:, :],
                        idxs_ap=idx_sb[:, t * SW:(t + 1) * SW],
                        num_idxs=NI, num_idxs_reg=reg_ni, elem_size=P)
                    oh6 = build_onehot(sp6, t, BF16, "oh6")
                    pg6 = pp6.tile([F5, P], F32, tag="pg6", name=f"pg6_{t}")
                    nc.tensor.matmul(out=pg6[:], lhsT=hres5[t][:, 0:F5],
                                     rhs=ident_b[:],
                                     start=True, stop=False,
                                     skip_group_check=True)
                    for c in range(C):
                        nc.tensor.matmul(out=pg6[:],
                                         lhsT=g6[:, c * P:c * P + F5],
                                         rhs=oh6[:, c * P:(c + 1) * P],
                                         start=False, stop=(c == C - 1),
                                         skip_group_check=True)
                    st7 = sp6.tile([F5, P], BF16, tag="st7", name=f"st7_{t}")
                    nc.scalar.activation(out=st7[:], in_=pg6[:], func=Ident)
                    pout = po6.tile([P, FO], F32, tag="pout", name=f"po_{t}")
                    nc.tensor.matmul(out=pout[:], lhsT=st7[:], rhs=W6_sb[:],
                                     start=True, stop=True)
                    os_ = sp6.tile([P, FO], F32, tag="os", name=f"o_{t}")
                    nc.vector.tensor_add(out=os_[:], in0=pout[:], in1=b6r_sb[:])
                    nc.sync.dma_start(out=out_d[t * P:(t + 1) * P, :], in_=os_[:])

    if split_mw:
        _split_multiwaits(nc)
    nc.compile()
    return nc


def _prepare(batch_vertices, img_features, edge_indices,
             W1, b1, W2, b2, W3, b3, W4, b4, W5, b5, W6, b6):
    B, N, _ = batch_vertices.shape
    FM = W1.shape[1]
    F5 = W5.shape[1]
    FO = W6.shape[1]

    ei = np.asarray(edge_indices).astype(np.int64)
    g = _pack_graph(ei[0], ei[1], N)
    NP, T, C, SW, perm = g["NP"], g["T"], g["C"], g["SW"], g["perm"]

    KM = FM // P
    hc = img_features.astype(np.float32) @ W1[3:].astype(np.float32)

    valid = perm >= 0
    vperm = np.zeros((B, NP, 3), np.float32)
    vperm[:, valid, :] = batch_vertices[:, perm[valid], :]

    common = {
        "W1v": np.ascontiguousarray(W1[:3]).astype(BF),
        "W2": np.ascontiguousarray(W2).astype(BF),
        "W3": np.ascontiguousarray(W3).astype(BF),
        "W4": np.ascontiguousarray(W4).astype(BF),
        "W5": np.ascontiguousarray(W5).astype(BF),
        "W6": np.ascontiguousarray(W6).astype(BF),
        "B14": np.ascontiguousarray(
            np.stack([b.reshape(KM, P).T for b in (b1, b2, b3, b4)],
                     axis=1).reshape(P, 4 * KM).astype(np.float32)),
        "b5col": np.ascontiguousarray(b5.astype(np.float32).reshape(F5, 1)),
        "b6rep": np.tile(b6.astype(np.float32), (P, 1)),
        "ones1": np.ones((1, P), BF),
        "idx16": g["idx16"], "slotb": g["slot"], "normb": g["norm"],
        "dinv2": g["dinv2"],
    }
    in_maps = []
    for b in range(B):
        m = dict(common)
        m["xT1"] = np.ascontiguousarray(vperm[b].T).astype(BF)
        m["hcrow"] = hc[b].reshape(1, FM).astype(BF)
        in_maps.append(m)
    meta = dict(NP=NP, T=T, C=C, SW=SW, perm=perm, valid=valid, B=B, N=N,
                FM=FM, F5=F5, FO=FO)
    return in_maps, meta


_BUILD_CACHE = {}


def run(inputs, trace=False):
    in_maps, meta = _prepare(**inputs)
    key = (meta["NP"], meta["C"], meta["FM"], meta["F5"], meta["FO"])
    if key not in _BUILD_CACHE:
        t0 = time.time()
        _BUILD_CACHE[key] = _build_nc(meta["NP"], meta["T"], meta["C"],
                                      meta["SW"], meta["FM"], meta["F5"],
                                      meta["FO"])
        print(f"[kernel] built bass program in {time.time()-t0:.1f}s", file=sys.stderr)
    nc = _BUILD_CACHE[key]
    B = meta["B"]
    res = run_bass_kernel_spmd(nc, in_maps, core_ids=list(range(B)), trace=trace)
    perm, valid, N = meta["perm"], meta["valid"], meta["N"]
    out = np.empty((B, N, meta["FO"]), np.float32)
    for b in range(B):
        dev = res.results[b]["out"]
        out[b, perm[valid], :] = dev[valid, :]
    return out, res


def kernel(**inputs) -> np.ndarray:
    out, _ = run(inputs)
    return out


# revision 17
# speedup vs baseline: 1.8518x; 1.3868x over previous
"""GCN (6-layer GCNConv) Trainium2 Bass kernel — v3.

Data-parallel over batch (1 mesh per NeuronCore). Per layer
out = A_hat @ (x @ W) + b with A_hat = D^-1/2 (A+I) D^-1/2 shared across
batch and layers.

v3 structure (vs v2):
  - bf16 datapath: all matmul operands bf16 (PE 1 cyc/row vs 4 for f32),
    f32 PSUM accumulation; gather tables bf16 (half the HBM traffic).
  - One 768-index dma_gather per dst tile replaces 6 indirect DMAs
    (~6x less Pool-engine SWDGE descriptor-generation time).
  - h tables are stored pre-scaled by dinv^2, so the self-loop term is a
    single identity matmul from the SBUF-resident copy of the tile (no
    DRAM re-read, no DVE scaling); the per-edge norm becomes
    dinv[dst]/dinv[src].
  - Bias (+ReLU) folded into the feature-major PSUM->SBUF activation
    copies as per-partition bias APs; layer-1 image-feature term as a
    rank-1 matmul. Steady-state DVE work is only the one-hot builds.
  - Layers 5/6 (64-wide) keep their tables padded to 128 columns so the
    gather rows stay 256B-aligned.
"""
import sys
import time

sys.path.insert(0, "/opt/trn_rl_repo")
import numpy as np
import ml_dtypes
from contextlib import ExitStack

import concourse.bass as bass
import concourse.bacc as bacc
import concourse.mybir as mybir
import concourse.tile as tile
from concourse.bass_utils import run_bass_kernel_spmd
from concourse.masks import make_identity

P = 128
F32 = mybir.dt.float32
BF16 = mybir.dt.bfloat16
I16 = mybir.dt.int16
BF = ml_dtypes.bfloat16

_msw_ctr = [0]


def _split_multiwaits(nc, max_waits=1):
    """This walrus build rejects >1 sync wait per instruction: split extras
    onto preceding same-engine NOPs."""
    for f in nc.m.functions:
        for b in f.blocks:
            out, changed = [], False
            for inst in b.instructions:
                si = getattr(inst, "sync_info", None)
                waits = list(si.on_wait) if si is not None else []
                if len(waits) > max_waits:
                    changed = True
                    for w in waits[:-max_waits]:
                        _msw_ctr[0] += 1
                        nop = mybir.InstNoOp(name=f"msw-{_msw_ctr[0]}", ins=[], outs=[])
                        nop.engine = inst.engine
                        nop.sync_info = mybir.SyncInfo(on_wait=[w], on_update=[])
                        out.append(nop)
                    si.on_wait = waits[-max_waits:]
                out.append(inst)
            if changed:
                b.instructions = out
    return nc


def _pack_graph(src, dst, N):
    """Relabel nodes into degree-balanced 128-node tiles (no self-loops in
    the edge list). Edge normalization is expressed relative to tables that
    store h*dinv^2: message weight = dinv[dst]/dinv[src]."""
    T = (N + P - 1) // P
    NP = T * P
    indeg = np.bincount(dst, minlength=N)          # no-loop in-degree
    C = max(1, int(np.ceil(len(src) / (T * P))))

    order = np.argsort(-indeg, kind="stable")
    while True:
        cap = C * P
        load = np.zeros(T, np.int64)
        count = np.zeros(T, np.int64)
        assign = np.empty(N, np.int64)
        ok = True
        for v in order:
            d = int(indeg[v])
            best_t, best_rem = -1, -1
            for t in range(T):
                if count[t] < P:
                    rem = cap - load[t]
                    if rem > best_rem:
                        best_rem, best_t = rem, t
            if best_t < 0 or load[best_t] + d > cap:
                ok = False
                break
            assign[v] = best_t
            load[best_t] += d
            count[best_t] += 1
        if ok:
            break
        C += 1

    perm = np.full(NP, -1, np.int64)
    new_of_old = np.empty(N, np.int64)
    cursor = np.zeros(T, np.int64)
    for v in range(N):
        t = assign[v]
        nid = t * P + cursor[t]
        cursor[t] += 1
        perm[nid] = v
        new_of_old[v] = nid

    # symmetric normalization (degree INCLUDES self-loops, per GCN)
    deg = (indeg + 1).astype(np.float32)
    dinv = (1.0 / np.sqrt(deg, dtype=np.float32)).astype(np.float32)
    # gathered message rows already carry dinv[src]^2
    norm = (dinv[dst] / dinv[src]).astype(np.float32)

    src_n = new_of_old[src]
    dst_n = new_of_old[dst]
    tile_of_e = dst_n // P
    order_e = np.argsort(tile_of_e, kind="stable")
    src_n, dst_n, norm = src_n[order_e], dst_n[order_e], norm[order_e]
    tile_of_e = tile_of_e[order_e]

    gsrc = np.zeros((T, C, P), np.int32)
    slot = np.zeros((T, C, P), np.float32)
    nrm = np.zeros((T, C, P), np.float32)
    starts = np.searchsorted(tile_of_e, np.arange(T + 1))
    for t in range(T):
        lo, hi = starts[t], starts[t + 1]
        n_e = hi - lo
        assert n_e <= C * P, (t, n_e, C * P)
        fs = np.zeros(C * P, np.int32)
        fl = np.zeros(C * P, np.float32)
        fn = np.zeros(C * P, np.float32)
        fs[:n_e] = src_n[lo:hi]
        fl[:n_e] = (dst_n[lo:hi] - t * P).astype(np.float32)
        fn[:n_e] = norm[lo:hi]
        gsrc[t] = fs.reshape(C, P)
        slot[t] = fl.reshape(C, P)
        nrm[t] = fn.reshape(C, P)

    # int16 index table for dma_gather: flat slot i = c*128 + j of tile t
    # lives at [i % 16, t*SW + i//16]
    SW = C * P // 16
    idx16 = np.zeros((P, T * SW), np.int16)
    for t in range(T):
        flat = gsrc[t].reshape(C * P).astype(np.int16)   # i = c*128 + j
        # wrapped [i%16, i//16], replicated to every 16-partition stripe
        # (each GPSIMD cpu reads indices from its own stripe)
        idx16[:, t * SW:(t + 1) * SW] = np.tile(flat.reshape(SW, 16).T, (8, 1))

    # per-(slot, tile) dinv^2 used to pre-scale h tables (0 for dummies)
    dinv_new = np.zeros(NP, np.float32)
    valid = perm >= 0
    dinv_new[valid] = dinv[perm[valid]]
    dinv2 = (dinv_new ** 2).reshape(T, P).T.copy()   # [128, T]

    def dev(a):
        return np.ascontiguousarray(a.transpose(2, 0, 1).reshape(P, -1))

    return dict(NP=NP, T=T, C=C, SW=SW, perm=perm,
                dinv2=np.ascontiguousarray(dinv2), idx16=idx16,
                slot=dev(slot).astype(BF), norm=dev(nrm).astype(BF))


def _build_nc(NP, T, C, SW, FM, F5, FO, split_mw=False):
    import os
    scratch = int(os.environ.get("KBASS_SCRATCH", "32768"))
    # Bacc: compile() auto-inserts GPSIMD library loads (dma_gather needs the
    # 'mlp' ucode) and lowers pseudo-instructions to valid ISA.
    nc = bacc.Bacc("TRN2", dynamic_dma_scratch_size=scratch,
                   num_swdge_queues=4)
    TC = T * C
    KM = FM // P
    NI = C * P   # gather indices per dst tile

    d = {}
    d["xT1"] = nc.dram_tensor("xT1", [3, NP], BF16, kind="ExternalInput")
    d["hcrow"] = nc.dram_tensor("hcrow", [1, FM], BF16, kind="ExternalInput")
    d["W1v"] = nc.dram_tensor("W1v", [3, FM], BF16, kind="ExternalInput")
    for i in (2, 3, 4):
        d[f"W{i}"] = nc.dram_tensor(f"W{i}", [FM, FM], BF16, kind="ExternalInput")
    d["W5"] = nc.dram_tensor("W5", [FM, F5], BF16, kind="ExternalInput")
    d["W6"] = nc.dram_tensor("W6", [F5, FO], BF16, kind="ExternalInput")
    d["B14"] = nc.dram_tensor("B14", [P, 4 * KM], F32, kind="ExternalInput")
    d["b5col"] = nc.dram_tensor("b5col", [F5, 1], F32, kind="ExternalInput")
    d["b6rep"] = nc.dram_tensor("b6rep", [P, FO], F32, kind="ExternalInput")
    d["ones1"] = nc.dram_tensor("ones1", [1, P], BF16, kind="ExternalInput")
    d["idx16"] = nc.dram_tensor("idx16", [P, T * SW], I16, kind="ExternalInput")
    d["slotb"] = nc.dram_tensor("slotb", [P, TC], BF16, kind="ExternalInput")
    d["normb"] = nc.dram_tensor("normb", [P, TC], BF16, kind="ExternalInput")
    d["dinv2"] = nc.dram_tensor("dinv2", [P, T], F32, kind="ExternalInput")
    out_d = nc.dram_tensor("out", [NP, FO], F32, kind="ExternalOutput")

    h512 = [nc.dram_tensor(f"h{i}", [NP, FM], BF16, kind="Internal")
            for i in (1, 2, 3, 4)]
    h5t = nc.dram_tensor("h5t", [NP, P], BF16, kind="Internal")
    x6t = nc.dram_tensor("x6t", [NP, P], BF16, kind="Internal")

    Ident = mybir.ActivationFunctionType.Identity
    Relu = mybir.ActivationFunctionType.Relu

    with tile.TileContext(nc) as tc:
        with ExitStack() as ctx:
            res = ctx.enter_context(tc.tile_pool(name="res", bufs=1))
            idx_sb = res.tile([P, T * SW], I16)
            slot_sb = res.tile([P, TC], BF16)
            norm_sb = res.tile([P, TC], BF16)
            B14_sb = res.tile([P, 4 * KM], F32)
            dinv2_sb = res.tile([P, T], F32)
            b5c_sb = res.tile([F5, 1], F32)
            b6r_sb = res.tile([P, FO], F32)
            ones_sb = res.tile([1, P], BF16)
            hcrow_sb = res.tile([1, FM], BF16)
            for name, t_sb in [("idx16", idx_sb), ("slotb", slot_sb),
                               ("normb", norm_sb), ("B14", B14_sb),
                               ("dinv2", dinv2_sb), ("b5col", b5c_sb),
                               ("b6rep", b6r_sb), ("ones1", ones_sb),
                               ("hcrow", hcrow_sb)]:
                nc.sync.dma_start(out=t_sb[:], in_=d[name][:, :])
            reg_ni = nc.gpsimd.to_reg(NI)   # shared gather index-count register
            qctr = [0]   # global gather counter; keeps queue in lock-step
                         # with the scheduler's DMASW lane rotation (mod 8)
            iota_i = res.tile([P, P], mybir.dt.int32)
            nc.gpsimd.iota(iota_i[:], pattern=[[1, P]], base=0, channel_multiplier=0)
            iota_b = res.tile([P, P], BF16)
            nc.vector.tensor_copy(out=iota_b[:], in_=iota_i[:])
            ident_b = res.tile([P, P], BF16)
            make_identity(nc, ident_b[:])
            hres = [res.tile([P, FM], BF16, name=f"hres_{t}") for t in range(T)]
            # hres5 tiles are padded to 128 cols so the 64-wide tables keep
            # 256B gather rows; pad stays zero and is never computed on.
            hres5 = [res.tile([P, P], BF16, name=f"hres5_{t}") for t in range(T)]
            for t in range(T):
                nc.vector.memset(hres5[t][:, F5:P], 0.0)

            def build_onehot(sp, t, dt_, tag):
                oh = sp.tile([P, NI], dt_, tag=tag, name=f"oh_{tag}_{t}")
                oh3 = oh[:].rearrange("p (c j) -> p c j", c=C)
                nc.vector.tensor_tensor(
                    out=oh3,
                    in0=slot_sb[:, t * C:(t + 1) * C]
                        .rearrange("p (c u) -> p c u", u=1).to_broadcast([P, C, P]),
                    in1=iota_b[:].rearrange("p (u j) -> p u j", u=1)
                        .to_broadcast([P, C, P]),
                    op=mybir.AluOpType.is_equal,
                )
                nc.vector.tensor_tensor(
                    out=oh3, in0=oh3,
                    in1=norm_sb[:, t * C:(t + 1) * C]
                        .rearrange("p (c u) -> p c u", u=1).to_broadcast([P, C, P]),
                    op=mybir.AluOpType.mult,
                )
                return oh

            # ---- layer 1 dense: h1 = (verts @ W1[:3] + img@W1[3:]) * dinv2 ----
            with tc.tile_pool(name="l1", bufs=1) as l1p, \
                 tc.tile_pool(name="l1ps", bufs=2, space="PSUM") as l1ps:
                xT1_sb = l1p.tile([3, NP], BF16)
                nc.sync.dma_start(out=xT1_sb[:], in_=d["xT1"][:, :])
                W1v_sb = l1p.tile([3, FM], BF16)
                nc.sync.dma_start(out=W1v_sb[:], in_=d["W1v"][:, :])
                for t in range(T):
                    pd1 = l1ps.tile([P, FM], F32, tag="pd1", name=f"pd1_{t}")
                    nc.tensor.matmul(out=pd1[:], lhsT=xT1_sb[:, t * P:(t + 1) * P],
                                     rhs=W1v_sb[:], start=True, stop=False)
                    nc.tensor.matmul(out=pd1[:], lhsT=ones_sb[:], rhs=hcrow_sb[:],
                                     start=False, stop=True)
                    nc.scalar.activation(out=hres[t][:], in_=pd1[:], func=Ident,
                                         scale=dinv2_sb[:, t:t + 1])
                    nc.sync.dma_start(out=h512[0][t * P:(t + 1) * P, :],
                                      in_=hres[t][:])

            # ---- merged phases: scatter(i) + dense(i+1), i = 1..4 ----
            import os as _os
            SBUFS = int(_os.environ.get("KBASS_BUFS", "3"))
            for i in (1, 2, 3, 4):
                relu = i in (2, 4)
                h_src = h512[i - 1]
                F_out = FM if i < 4 else F5
                W_d = d[f"W{i + 1}"]
                with tc.tile_pool(name=f"ph{i}", bufs=SBUFS) as sp, \
                     tc.tile_pool(name=f"ph{i}w", bufs=1) as wp, \
                     tc.tile_pool(name=f"ph{i}ps", bufs=2, space="PSUM") as pp, \
                     tc.tile_pool(name=f"ph{i}pt", bufs=2, space="PSUM") as pt, \
                     tc.tile_pool(name=f"ph{i}pd", bufs=2, space="PSUM") as pd:
                    W_sb = [wp.tile([P, F_out], BF16, tag=f"w{k}", name=f"w{i}_{k}")
                            for k in range(KM)]
                    for k in range(KM):
                        nc.sync.dma_start(out=W_sb[k][:], in_=W_d[k * P:(k + 1) * P, :])
                    for t in range(T):
                        g = sp.tile([P, C * FM], BF16, tag="g", name=f"g{i}_{t}")
                        nc.gpsimd.dma_gather(
                            out_ap=g[:].rearrange("p (c f) -> p c f", c=C),
                            in_ap=h_src[:, :],
                            idxs_ap=idx_sb[:, t * SW:(t + 1) * SW],
                            num_idxs=NI, num_idxs_reg=reg_ni, elem_size=FM,
                            queue_num=qctr[0] % 4)
                        qctr[0] += 1
                        oh = build_onehot(sp, t, BF16, "oh")
                        pa = pp.tile([P, FM], F32, tag="pa", name=f"pa{i}_{t}")
                        for c in range(C):
                            nc.tensor.matmul(
                                out=pa[:], lhsT=oh[:, c * P:(c + 1) * P],
                                rhs=g[:, c * FM:(c + 1) * FM],
                                start=(c == 0), stop=(c == C - 1))
                        # self-loop (h rows pre-scaled by dinv^2) fused into
                        # the PSUM->SBUF copy on DVE
                        node = sp.tile([P, FM], BF16, tag="node", name=f"nd{i}_{t}")
                        nc.vector.tensor_add(out=node[:], in0=pa[:],
                                             in1=hres[t][:])
                        ptr = pt.tile([P, FM], F32, tag="ptr", name=f"pt{i}_{t}")
                        stage = sp.tile([P, FM], BF16, tag="stage", name=f"st{i}_{t}")
                        for fo in range(KM):
                            # feature-major transpose as node_chunk^T @ I
                            nc.tensor.matmul(
                                out=ptr[:, fo * P:(fo + 1) * P],
                                lhsT=node[:, fo * P:(fo + 1) * P],
                                rhs=ident_b[:],
                                start=True, stop=True)
                            nc.scalar.activation(
                                out=stage[:, fo * P:(fo + 1) * P],
                                in_=ptr[:, fo * P:(fo + 1) * P],
                                func=Relu if relu else Ident,
                                bias=B14_sb[:, (i - 1) * KM + fo:(i - 1) * KM + fo + 1])
                        pdn = pd.tile([P, F_out], F32, tag="pdn", name=f"pd{i}_{t}")
                        for k in range(KM):
                            nc.tensor.matmul(out=pdn[:],
                                             lhsT=stage[:, k * P:(k + 1) * P],
                                             rhs=W_sb[k][:], start=(k == 0),
                                             stop=(k == KM - 1))
                        if i < 4:
                            nc.scalar.activation(out=hres[t][:], in_=pdn[:],
                                                 func=Ident,
                                                 scale=dinv2_sb[:, t:t + 1])
                            nc.sync.dma_start(out=h512[i][t * P:(t + 1) * P, :],
                                              in_=hres[t][:])
                        else:
                            nc.scalar.activation(out=hres5[t][:, 0:F5],
                                                 in_=pdn[:], func=Ident,
                                                 scale=dinv2_sb[:, t:t + 1])
                            nc.sync.dma_start(out=h5t[t * P:(t + 1) * P, :],
                                              in_=hres5[t][:])

            # ---- phase 5: x6 = (A_hat h5 + b5) * dinv2, 64-wide ----
            with tc.tile_pool(name="s5", bufs=SBUFS) as sp5, \
                 tc.tile_pool(name="s5ps", bufs=2, space="PSUM") as pp5, \
                 tc.tile_pool(name="s5pt", bufs=2, space="PSUM") as pt5:
                for t in range(T):
                    g5 = sp5.tile([P, C * P], BF16, tag="g5", name=f"g5_{t}")
                    nc.gpsimd.dma_gather(
                        out_ap=g5[:].rearrange("p (c f) -> p c f", c=C),
                        in_ap=h5t[:, :],
                        idxs_ap=idx_sb[:, t * SW:(t + 1) * SW],
                        num_idxs=NI, num_idxs_reg=reg_ni, elem_size=P,
                        queue_num=qctr[0] % 4)
                    qctr[0] += 1
                    oh5 = build_onehot(sp5, t, BF16, "oh5")
                    pg5 = pp5.tile([F5, P], F32, tag="pg5", name=f"pg5_{t}")
                    nc.tensor.matmul(out=pg5[:], lhsT=hres5[t][:, 0:F5],
                                     rhs=ident_b[:],
                                     start=True, stop=False,
                                     skip_group_check=True)
                    for c in range(C):
                        nc.tensor.matmul(out=pg5[:],
                                         lhsT=g5[:, c * P:c * P + F5],
                                         rhs=oh5[:, c * P:(c + 1) * P],
                                         start=False, stop=(c == C - 1),
                                         skip_group_check=True)
                    st6 = sp5.tile([F5, P], BF16, tag="st6", name=f"st6_{t}")
                    nc.scalar.activation(out=st6[:], in_=pg5[:], func=Ident,
                                         bias=b5c_sb[:, 0:1])
                    pt6 = pt5.tile([P, F5], F32, tag="pt6", name=f"pt6_{t}")
                    nc.tensor.matmul(out=pt6[:], lhsT=st6[:],
                                     rhs=ident_b[0:F5, 0:F5],
                                     start=True, stop=True)
                    nc.scalar.activation(out=hres5[t][:, 0:F5], in_=pt6[:],
                                         func=Ident,
                                         scale=dinv2_sb[:, t:t + 1])
                    nc.sync.dma_start(out=x6t[t * P:(t + 1) * P, :],
                                      in_=hres5[t][:])

            # ---- phase 6: out = (A_hat x6) @ W6 + b6 ----
            with tc.tile_pool(name="s6", bufs=SBUFS) as sp6, \
                 tc.tile_pool(name="s6w", bufs=1) as wp6, \
                 tc.tile_pool(name="s6ps", bufs=2, space="PSUM") as pp6, \
                 tc.tile_pool(name="s6pd", bufs=2, space="PSUM") as po6:
                W6_sb = wp6.tile([F5, FO], BF16)
                nc.sync.dma_start(out=W6_sb[:], in_=d["W6"][:, :])
                for t in range(T):
                    g6 = sp6.tile([P, C * P], BF16, tag="g6", name=f"g6_{t}")
                    nc.gpsimd.dma_gather(
                        out_ap=g6[:].rearrange("p (c f) -> p c f", c=C),
                        in_ap=x6t[:, :],
                        idxs_ap=idx_sb[:, t * SW:(t + 1) * SW],
                        num_idxs=NI, num_idxs_reg=reg_ni, elem_size=P,
                        queue_num=qctr[0] % 4)
                    qctr[0] += 1
                    oh6 = build_onehot(sp6, t, BF16, "oh6")
                    pg6 = pp6.tile([F5, P], F32, tag="pg6", name=f"pg6_{t}")
                    nc.tensor.matmul(out=pg6[:], lhsT=hres5[t][:, 0:F5],
                                     rhs=ident_b[:],
                                     start=True, stop=False,
                                     skip_group_check=True)
                    for c in range(C):
                        nc.tensor.matmul(out=pg6[:],
                                         lhsT=g6[:, c * P:c * P + F5],
                                         rhs=oh6[:, c * P:(c + 1) * P],
                                         start=False, stop=(c == C - 1),
                                         skip_group_check=True)
                    st7 = sp6.tile([F5, P], BF16, tag="st7", name=f"st7_{t}")
                    nc.scalar.activation(out=st7[:], in_=pg6[:], func=Ident)
                    pout = po6.tile([P, FO], F32, tag="pout", name=f"po_{t}")
                    nc.tensor.matmul(out=pout[:], lhsT=st7[:], rhs=W6_sb[:],
                                     start=True, stop=True)
                    os_ = sp6.tile([P, FO], F32, tag="os", name=f"o_{t}")
                    nc.vector.tensor_add(out=os_[:], in0=pout[:], in1=b6r_sb[:])
                    nc.sync.dma_start(out=out_d[t * P:(t + 1) * P, :], in_=os_[:])

    if split_mw:
        _split_multiwaits(nc)
    nc.compile()
    return nc


def _prepare(batch_vertices, img_features, edge_indices,
             W1, b1, W2, b2, W3, b3, W4, b4, W5, b5, W6, b6):
    B, N, _ = batch_vertices.shape
    FM = W1.shape[1]
    F5 = W5.shape[1]
    FO = W6.shape[1]

    ei = np.asarray(edge_indices).astype(np.int64)
    g = _pack_graph(ei[0], ei[1], N)
    NP, T, C, SW, perm = g["NP"], g["T"], g["C"], g["SW"], g["perm"]

    KM = FM // P
    hc = img_features.astype(np.float32) @ W1[3:].astype(np.float32)

    valid = perm >= 0
    vperm = np.zeros((B, NP, 3), np.float32)
    vperm[:, valid, :] = batch_vertices[:, perm[valid], :]

    common = {
        "W1v": np.ascontiguousarray(W1[:3]).astype(BF),
        "W2": np.ascontiguousarray(W2).astype(BF),
        "W3": np.ascontiguousarray(W3).astype(BF),
        "W4": np.ascontiguousarray(W4).astype(BF),
        "W5": np.ascontiguousarray(W5).astype(BF),
        "W6": np.ascontiguousarray(W6).astype(BF),
        "B14": np.ascontiguousarray(
            np.stack([b.reshape(KM, P).T for b in (b1, b2, b3, b4)],
                     axis=1).reshape(P, 4 * KM).astype(np.float32)),
        "b5col": np.ascontiguousarray(b5.astype(np.float32).reshape(F5, 1)),
        "b6rep": np.tile(b6.astype(np.float32), (P, 1)),
        "ones1": np.ones((1, P), BF),
        "idx16": g["idx16"], "slotb": g["slot"], "normb": g["norm"],
        "dinv2": g["dinv2"],
    }
    in_maps = []
    for b in range(B):
        m = dict(common)
        m["xT1"] = np.ascontiguousarray(vperm[b].T).astype(BF)
        m["hcrow"] = hc[b].reshape(1, FM).astype(BF)
        in_maps.append(m)
    meta = dict(NP=NP, T=T, C=C, SW=SW, perm=perm, valid=valid, B=B, N=N,
                FM=FM, F5=F5, FO=FO)
    return in_maps, meta


_BUILD_CACHE = {}


def run(inputs, trace=False):
    in_maps, meta = _prepare(**inputs)
    key = (meta["NP"], meta["C"], meta["FM"], meta["F5"], meta["FO"])
    if key not in _BUILD_CACHE:
        t0 = time.time()
        _BUILD_CACHE[key] = _build_nc(meta["NP"], meta["T"], meta["C"],
                                      meta["SW"], meta["FM"], meta["F5"],
                                      meta["FO"])
        print(f"[kernel] built bass program in {time.time()-t0:.1f}s", file=sys.stderr)
    nc = _BUILD_CACHE[key]
    B = meta["B"]
    res = run_bass_kernel_spmd(nc, in_maps, core_ids=list(range(B)), trace=trace)
    perm, valid, N = meta["perm"], meta["valid"], meta["N"]
    out = np.empty((B, N, meta["FO"]), np.float32)
    for b in range(B):
        dev = res.results[b]["out"]
        out[b, perm[valid], :] = dev[valid, :]
    return out, res


def kernel(**inputs) -> np.ndarray:
    out, _ = run(inputs)
    return out


# revision 18
# speedup vs baseline: 1.8746x; 1.0123x over previous
"""GCN (6-layer GCNConv) Trainium2 Bass kernel — v5.

Data-parallel over batch (1 mesh per NeuronCore). Per layer
out = A_hat @ (x @ W) + b with A_hat = D^-1/2 (A+I) D^-1/2 shared across
batch and layers.

v5 structure:
  - bf16 datapath (PE 1 cyc/row), f32 PSUM accumulation.
  - One 768-index dma_gather per dst tile (indices int16, wrapped in 16
    partitions and replicated to all eight 16-partition stripes — each
    GPSIMD cpu reads its own stripe). Gathers round-robin over 4 SWDGE
    queues in lock-step with the scheduler's DMASW lane rotation.
  - h tables pre-scaled by dinv^2; self-loop + PSUM drain fused into one
    DVE add (node = pa + hres).
  - One-hot scatter matrices precomputed on the host (norm-scaled bf16)
    and streamed from DRAM per tile — no DVE one-hot builds.
  - Bias via a reserved gather slot per tile: its index points at a bias
    row appended to each h table and its one-hot row is all-ones, so the
    segment-sum matmul adds the layer bias for free.
  - Per-tile PSUM->SBUF stage copy is a single wide activation (ReLU
    folded where the reference has it).
"""
import sys
import time

sys.path.insert(0, "/opt/trn_rl_repo")
import numpy as np
import ml_dtypes
from contextlib import ExitStack

import concourse.bass as bass
import concourse.bacc as bacc
import concourse.mybir as mybir
import concourse.tile as tile
from concourse.bass_utils import run_bass_kernel_spmd
from concourse.masks import make_identity

P = 128
F32 = mybir.dt.float32
BF16 = mybir.dt.bfloat16
I16 = mybir.dt.int16
BF = ml_dtypes.bfloat16


def _pack_graph(src, dst, N):
    """Relabel nodes into degree-balanced 128-node tiles. Every tile
    reserves one gather slot (the 'bias slot'): its index points at the
    bias row (row NP) of the gather table and its one-hot row is all-ones.
    Edge norm is dinv[dst]/dinv[src] (tables store h*dinv^2)."""
    T = (N + P - 1) // P
    NP = T * P
    indeg = np.bincount(dst, minlength=N)          # no-loop in-degree
    C = max(1, int(np.ceil((len(src) + T) / (T * P))))

    order = np.argsort(-indeg, kind="stable")
    while True:
        cap = C * P - 1                             # reserve the bias slot
        load = np.zeros(T, np.int64)
        count = np.zeros(T, np.int64)
        assign = np.empty(N, np.int64)
        ok = True
        for v in order:
            d = int(indeg[v])
            best_t, best_rem = -1, -1
            for t in range(T):
                if count[t] < P:
                    rem = cap - load[t]
                    if rem > best_rem:
                        best_rem, best_t = rem, t
            if best_t < 0 or load[best_t] + d > cap:
                ok = False
                break
            assign[v] = best_t
            load[best_t] += d
            count[best_t] += 1
        if ok:
            break
        C += 1

    perm = np.full(NP, -1, np.int64)
    new_of_old = np.empty(N, np.int64)
    cursor = np.zeros(T, np.int64)
    for v in range(N):
        t = assign[v]
        nid = t * P + cursor[t]
        cursor[t] += 1
        perm[nid] = v
        new_of_old[v] = nid

    # symmetric normalization (degree INCLUDES self-loops, per GCN)
    deg = (indeg + 1).astype(np.float32)
    dinv = (1.0 / np.sqrt(deg, dtype=np.float32)).astype(np.float32)
    norm = (dinv[dst] / dinv[src]).astype(np.float32)

    src_n = new_of_old[src]
    dst_n = new_of_old[dst]
    tile_of_e = dst_n // P
    order_e = np.argsort(tile_of_e, kind="stable")
    src_n, dst_n, norm = src_n[order_e], dst_n[order_e], norm[order_e]
    tile_of_e = tile_of_e[order_e]

    CP = C * P
    gsrc = np.zeros((T, CP), np.int32)
    ohT = np.zeros((P, T * CP), BF)    # [slot-in-chunk, t*CP + c*128 + dstoff]
    starts = np.searchsorted(tile_of_e, np.arange(T + 1))
    for t in range(T):
        lo, hi = starts[t], starts[t + 1]
        n_e = hi - lo
        assert n_e <= CP - 1, (t, n_e, CP)
        fs = np.zeros(CP, np.int32)
        fs[:n_e] = src_n[lo:hi]
        fs[n_e] = NP                                # bias slot -> bias row
        gsrc[t] = fs
        oh_t = np.zeros((CP, P), np.float32)        # [flat slot, dst offset]
        oh_t[np.arange(n_e), dst_n[lo:hi] - t * P] = norm[lo:hi]
        oh_t[n_e, :] = 1.0                          # all-ones bias row
        # flat slot i = c*128 + e ; device tile is [e, c*128 + j]
        ohT[:, t * CP:(t + 1) * CP] = (
            oh_t.reshape(C, P, P).transpose(1, 0, 2).reshape(P, CP).astype(BF))

    # int16 index table: wrapped [i%16, i//16], replicated to all 8 stripes
    SW = CP // 16
    idx16 = np.zeros((P, T * SW), np.int16)
    for t in range(T):
        flat = gsrc[t].astype(np.int16)
        idx16[:, t * SW:(t + 1) * SW] = np.tile(flat.reshape(SW, 16).T, (8, 1))

    dinv_new = np.zeros(NP, np.float32)
    valid = perm >= 0
    dinv_new[valid] = dinv[perm[valid]]
    dinv2 = (dinv_new ** 2).reshape(T, P).T.copy()   # [128, T]

    return dict(NP=NP, T=T, C=C, SW=SW, perm=perm,
                dinv2=np.ascontiguousarray(dinv2), idx16=idx16, ohT=ohT)


def _build_nc(NP, T, C, SW, FM, F5, FO):
    import os
    scratch = int(os.environ.get("KBASS_SCRATCH", "32768"))
    nc = bacc.Bacc("TRN2", dynamic_dma_scratch_size=scratch,
                   num_swdge_queues=4)
    KM = FM // P
    NI = C * P   # gather indices per dst tile (includes the bias slot)
    NR = NP + 1  # gather tables have a bias row at NP

    d = {}
    d["xT1"] = nc.dram_tensor("xT1", [3, NP], BF16, kind="ExternalInput")
    d["hcrow"] = nc.dram_tensor("hcrow", [1, FM], BF16, kind="ExternalInput")
    d["W1v"] = nc.dram_tensor("W1v", [3, FM], BF16, kind="ExternalInput")
    for i in (2, 3, 4):
        d[f"W{i}"] = nc.dram_tensor(f"W{i}", [FM, FM], BF16, kind="ExternalInput")
    d["W5"] = nc.dram_tensor("W5", [FM, F5], BF16, kind="ExternalInput")
    d["W6"] = nc.dram_tensor("W6", [F5, FO], BF16, kind="ExternalInput")
    d["Brows"] = nc.dram_tensor("Brows", [4, FM], BF16, kind="ExternalInput")
    d["Brows56"] = nc.dram_tensor("Brows56", [2, P], BF16, kind="ExternalInput")
    d["b6rep"] = nc.dram_tensor("b6rep", [P, FO], F32, kind="ExternalInput")
    d["idx16"] = nc.dram_tensor("idx16", [P, T * SW], I16, kind="ExternalInput")
    d["ohT"] = nc.dram_tensor("ohT", [P, T * NI], BF16, kind="ExternalInput")
    d["dinv2"] = nc.dram_tensor("dinv2", [P, T], F32, kind="ExternalInput")
    out_d = nc.dram_tensor("out", [NP, FO], F32, kind="ExternalOutput")

    h512 = [nc.dram_tensor(f"h{i}", [NR, FM], BF16, kind="Internal")
            for i in (1, 2, 3, 4)]
    h5t = nc.dram_tensor("h5t", [NR, P], BF16, kind="Internal")
    x6t = nc.dram_tensor("x6t", [NR, P], BF16, kind="Internal")

    Ident = mybir.ActivationFunctionType.Identity
    Relu = mybir.ActivationFunctionType.Relu

    with tile.TileContext(nc) as tc:
        with ExitStack() as ctx:
            res = ctx.enter_context(tc.tile_pool(name="res", bufs=1))
            idx_sb = res.tile([P, T * SW], I16)
            dinv2_sb = res.tile([P, T], F32)
            b6r_sb = res.tile([P, FO], F32)
            hcrow_sb = res.tile([1, FM], BF16)
            brows_sb = res.tile([4, FM], BF16)
            brows56_sb = res.tile([2, P], BF16)
            for name, t_sb in [("idx16", idx_sb), ("dinv2", dinv2_sb),
                               ("b6rep", b6r_sb), ("hcrow", hcrow_sb),
                               ("Brows", brows_sb), ("Brows56", brows56_sb)]:
                nc.sync.dma_start(out=t_sb[:], in_=d[name][:, :])
            reg_ni = nc.gpsimd.to_reg(NI)
            qctr = [0]   # keeps queue_num in lock-step with DMASW lanes
            ident_b = res.tile([P, P], BF16)
            make_identity(nc, ident_b[:])
            hres = [res.tile([P, FM], BF16, name=f"hres_{t}") for t in range(T)]
            hres5 = [res.tile([P, P], BF16, name=f"hres5_{t}") for t in range(T)]
            for t in range(T):
                nc.vector.memset(hres5[t][:, F5:P], 0.0)

            def gather(sp, h_src, t, width, tag):
                g = sp.tile([P, C * width], BF16, tag=tag, name=f"{tag}_{t}")
                nc.gpsimd.dma_gather(
                    out_ap=g[:].rearrange("p (c f) -> p c f", c=C),
                    in_ap=h_src[:, :],
                    idxs_ap=idx_sb[:, t * SW:(t + 1) * SW],
                    num_idxs=NI, num_idxs_reg=reg_ni, elem_size=width,
                    queue_num=qctr[0] % 4)
                qctr[0] += 1
                return g

            def load_oh(sp, t, tag):
                oh = sp.tile([P, NI], BF16, tag=tag, name=f"{tag}_{t}")
                nc.sync.dma_start(out=oh[:], in_=d["ohT"][:, t * NI:(t + 1) * NI])
                return oh

            # ---- layer 1 dense: h1 = (verts @ W1[:3] + img@W1[3:]) * dinv2 ----
            with tc.tile_pool(name="l1", bufs=1) as l1p, \
                 tc.tile_pool(name="l1ps", bufs=2, space="PSUM") as l1ps:
                nc.sync.dma_start(out=h512[0][NP:NP + 1, :], in_=brows_sb[0:1, :])
                xT1_sb = l1p.tile([3, NP], BF16)
                nc.sync.dma_start(out=xT1_sb[:], in_=d["xT1"][:, :])
                W1v_sb = l1p.tile([3, FM], BF16)
                nc.sync.dma_start(out=W1v_sb[:], in_=d["W1v"][:, :])
                ones1 = l1p.tile([1, P], BF16)
                nc.vector.memset(ones1[:], 1.0)
                for t in range(T):
                    pd1 = l1ps.tile([P, FM], F32, tag="pd1", name=f"pd1_{t}")
                    nc.tensor.matmul(out=pd1[:], lhsT=xT1_sb[:, t * P:(t + 1) * P],
                                     rhs=W1v_sb[:], start=True, stop=False)
                    nc.tensor.matmul(out=pd1[:], lhsT=ones1[:], rhs=hcrow_sb[:],
                                     start=False, stop=True)
                    nc.scalar.activation(out=hres[t][:], in_=pd1[:], func=Ident,
                                         scale=dinv2_sb[:, t:t + 1])
                    nc.sync.dma_start(out=h512[0][t * P:(t + 1) * P, :],
                                      in_=hres[t][:])

            # ---- merged phases: scatter(i) + dense(i+1), i = 1..4 ----
            import os as _os
            SBUFS = int(_os.environ.get("KBASS_BUFS", "3"))
            for i in (1, 2, 3, 4):
                relu = i in (2, 4)
                h_src = h512[i - 1]
                F_out = FM if i < 4 else F5
                W_d = d[f"W{i + 1}"]
                with tc.tile_pool(name=f"ph{i}", bufs=SBUFS) as sp, \
                     tc.tile_pool(name=f"ph{i}w", bufs=1) as wp, \
                     tc.tile_pool(name=f"ph{i}ps", bufs=2, space="PSUM") as pp, \
                     tc.tile_pool(name=f"ph{i}pt", bufs=2, space="PSUM") as pt, \
                     tc.tile_pool(name=f"ph{i}pd", bufs=2, space="PSUM") as pd:
                    # bias row of the NEXT phase's gather table
                    if i < 4:
                        nc.sync.dma_start(out=h512[i][NP:NP + 1, :],
                                          in_=brows_sb[i:i + 1, :])
                    else:
                        nc.sync.dma_start(out=h5t[NP:NP + 1, :],
                                          in_=brows56_sb[0:1, :])
                    W_sb = [wp.tile([P, F_out], BF16, tag=f"w{k}", name=f"w{i}_{k}")
                            for k in range(KM)]
                    for k in range(KM):
                        nc.sync.dma_start(out=W_sb[k][:], in_=W_d[k * P:(k + 1) * P, :])
                    for t in range(T):
                        g = gather(sp, h_src, t, FM, f"g{i}")
                        oh = load_oh(sp, t, f"oh{i}")
                        pa = pp.tile([P, FM], F32, tag="pa", name=f"pa{i}_{t}")
                        for c in range(C):
                            nc.tensor.matmul(
                                out=pa[:], lhsT=oh[:, c * P:(c + 1) * P],
                                rhs=g[:, c * FM:(c + 1) * FM],
                                start=(c == 0), stop=(c == C - 1))
                        # self-loop (h rows pre-scaled by dinv^2) fused into
                        # the PSUM->SBUF drain on DVE; bias came via the
                        # all-ones one-hot slot
                        node = sp.tile([P, FM], BF16, tag="node", name=f"nd{i}_{t}")
                        nc.vector.tensor_add(out=node[:], in0=pa[:],
                                             in1=hres[t][:])
                        ptr = pt.tile([P, FM], F32, tag="ptr", name=f"pt{i}_{t}")
                        for fo in range(KM):
                            nc.tensor.matmul(
                                out=ptr[:, fo * P:(fo + 1) * P],
                                lhsT=node[:, fo * P:(fo + 1) * P],
                                rhs=ident_b[:],
                                start=True, stop=True)
                        stage = sp.tile([P, FM], BF16, tag="stage", name=f"st{i}_{t}")
                        nc.scalar.activation(out=stage[:], in_=ptr[:],
                                             func=Relu if relu else Ident)
                        pdn = pd.tile([P, F_out], F32, tag="pdn", name=f"pd{i}_{t}")
                        for k in range(KM):
                            nc.tensor.matmul(out=pdn[:],
                                             lhsT=stage[:, k * P:(k + 1) * P],
                                             rhs=W_sb[k][:], start=(k == 0),
                                             stop=(k == KM - 1))
                        if i < 4:
                            nc.scalar.activation(out=hres[t][:], in_=pdn[:],
                                                 func=Ident,
                                                 scale=dinv2_sb[:, t:t + 1])
                            nc.sync.dma_start(out=h512[i][t * P:(t + 1) * P, :],
                                              in_=hres[t][:])
                        else:
                            nc.scalar.activation(out=hres5[t][:, 0:F5],
                                                 in_=pdn[:], func=Ident,
                                                 scale=dinv2_sb[:, t:t + 1])
                            nc.sync.dma_start(out=h5t[t * P:(t + 1) * P, :],
                                              in_=hres5[t][:])

            # ---- phase 5: x6 = (A_hat h5 + b5) * dinv2, 64-wide ----
            with tc.tile_pool(name="s5", bufs=SBUFS) as sp5, \
                 tc.tile_pool(name="s5ps", bufs=2, space="PSUM") as pp5, \
                 tc.tile_pool(name="s5pt", bufs=2, space="PSUM") as pt5:
                nc.sync.dma_start(out=x6t[NP:NP + 1, :], in_=brows56_sb[1:2, :])
                for t in range(T):
                    g5 = gather(sp5, h5t, t, P, "g5")
                    oh5 = load_oh(sp5, t, "oh5")
                    pg5 = pp5.tile([F5, P], F32, tag="pg5", name=f"pg5_{t}")
                    nc.tensor.matmul(out=pg5[:], lhsT=hres5[t][:, 0:F5],
                                     rhs=ident_b[:],
                                     start=True, stop=False,
                                     skip_group_check=True)
                    for c in range(C):
                        nc.tensor.matmul(out=pg5[:],
                                         lhsT=g5[:, c * P:c * P + F5],
                                         rhs=oh5[:, c * P:(c + 1) * P],
                                         start=False, stop=(c == C - 1),
                                         skip_group_check=True)
                    st6 = sp5.tile([F5, P], BF16, tag="st6", name=f"st6_{t}")
                    nc.scalar.activation(out=st6[:], in_=pg5[:], func=Ident)
                    pt6 = pt5.tile([P, F5], F32, tag="pt6", name=f"pt6_{t}")
                    nc.tensor.matmul(out=pt6[:], lhsT=st6[:],
                                     rhs=ident_b[0:F5, 0:F5],
                                     start=True, stop=True)
                    nc.scalar.activation(out=hres5[t][:, 0:F5], in_=pt6[:],
                                         func=Ident,
                                         scale=dinv2_sb[:, t:t + 1])
                    nc.sync.dma_start(out=x6t[t * P:(t + 1) * P, :],
                                      in_=hres5[t][:])

            # ---- phase 6: out = (A_hat x6) @ W6 + b6 ----
            with tc.tile_pool(name="s6", bufs=SBUFS) as sp6, \
                 tc.tile_pool(name="s6w", bufs=1) as wp6, \
                 tc.tile_pool(name="s6ps", bufs=2, space="PSUM") as pp6, \
                 tc.tile_pool(name="s6pd", bufs=2, space="PSUM") as po6:
                W6_sb = wp6.tile([F5, FO], BF16)
                nc.sync.dma_start(out=W6_sb[:], in_=d["W6"][:, :])
                for t in range(T):
                    g6 = gather(sp6, x6t, t, P, "g6")
                    oh6 = load_oh(sp6, t, "oh6")
                    pg6 = pp6.tile([F5, P], F32, tag="pg6", name=f"pg6_{t}")
                    nc.tensor.matmul(out=pg6[:], lhsT=hres5[t][:, 0:F5],
                                     rhs=ident_b[:],
                                     start=True, stop=False,
                                     skip_group_check=True)
                    for c in range(C):
                        nc.tensor.matmul(out=pg6[:],
                                         lhsT=g6[:, c * P:c * P + F5],
                                         rhs=oh6[:, c * P:(c + 1) * P],
                                         start=False, stop=(c == C - 1),
                                         skip_group_check=True)
                    st7 = sp6.tile([F5, P], BF16, tag="st7", name=f"st7_{t}")
                    nc.scalar.activation(out=st7[:], in_=pg6[:], func=Ident)
                    pout = po6.tile([P, FO], F32, tag="pout", name=f"po_{t}")
                    nc.tensor.matmul(out=pout[:], lhsT=st7[:], rhs=W6_sb[:],
                                     start=True, stop=True)
                    os_ = sp6.tile([P, FO], F32, tag="os", name=f"o_{t}")
                    nc.vector.tensor_add(out=os_[:], in0=pout[:], in1=b6r_sb[:])
                    nc.sync.dma_start(out=out_d[t * P:(t + 1) * P, :], in_=os_[:])

    nc.compile()
    return nc


def _prepare(batch_vertices, img_features, edge_indices,
             W1, b1, W2, b2, W3, b3, W4, b4, W5, b5, W6, b6):
    B, N, _ = batch_vertices.shape
    FM = W1.shape[1]
    F5 = W5.shape[1]
    FO = W6.shape[1]

    ei = np.asarray(edge_indices).astype(np.int64)
    g = _pack_graph(ei[0], ei[1], N)
    NP, T, C, SW, perm = g["NP"], g["T"], g["C"], g["SW"], g["perm"]

    hc = img_features.astype(np.float32) @ W1[3:].astype(np.float32)

    valid = perm >= 0
    vperm = np.zeros((B, NP, 3), np.float32)
    vperm[:, valid, :] = batch_vertices[:, perm[valid], :]

    b5row = np.zeros((1, P), np.float32)
    b5row[0, :F5] = b5
    common = {
        "W1v": np.ascontiguousarray(W1[:3]).astype(BF),
        "W2": np.ascontiguousarray(W2).astype(BF),
        "W3": np.ascontiguousarray(W3).astype(BF),
        "W4": np.ascontiguousarray(W4).astype(BF),
        "W5": np.ascontiguousarray(W5).astype(BF),
        "W6": np.ascontiguousarray(W6).astype(BF),
        "Brows": np.stack([b1, b2, b3, b4]).astype(BF),
        "Brows56": np.concatenate([b5row, np.zeros((1, P), np.float32)]).astype(BF),
        "b6rep": np.tile(b6.astype(np.float32), (P, 1)),
        "idx16": g["idx16"], "ohT": g["ohT"], "dinv2": g["dinv2"],
    }
    in_maps = []
    for b in range(B):
        m = dict(common)
        m["xT1"] = np.ascontiguousarray(vperm[b].T).astype(BF)
        m["hcrow"] = hc[b].reshape(1, FM).astype(BF)
        in_maps.append(m)
    meta = dict(NP=NP, T=T, C=C, SW=SW, perm=perm, valid=valid, B=B, N=N,
                FM=FM, F5=F5, FO=FO)
    return in_maps, meta


_BUILD_CACHE = {}


def run(inputs, trace=False):
    in_maps, meta = _prepare(**inputs)
    key = (meta["NP"], meta["C"], meta["FM"], meta["F5"], meta["FO"])
    if key not in _BUILD_CACHE:
        t0 = time.time()
        _BUILD_CACHE[key] = _build_nc(meta["NP"], meta["T"], meta["C"],
                                      meta["SW"], meta["FM"], meta["F5"],
                                      meta["FO"])
        print(f"[kernel] built bass program in {time.time()-t0:.1f}s", file=sys.stderr)
    nc = _BUILD_CACHE[key]
    B = meta["B"]
    res = run_bass_kernel_spmd(nc, in_maps, core_ids=list(range(B)), trace=trace)
    perm, valid, N = meta["perm"], meta["valid"], meta["N"]
    out = np.empty((B, N, meta["FO"]), np.float32)
    for b in range(B):
        dev = res.results[b]["out"]
        out[b, perm[valid], :] = dev[valid, :]
    return out, res


def kernel(**inputs) -> np.ndarray:
    out, _ = run(inputs)
    return out


# revision 20
# speedup vs baseline: 2.2261x; 1.1875x over previous
"""GCN (6-layer GCNConv) Trainium2 Bass kernel — v5.

Data-parallel over batch (1 mesh per NeuronCore). Per layer
out = A_hat @ (x @ W) + b with A_hat = D^-1/2 (A+I) D^-1/2 shared across
batch and layers.

v5 structure:
  - bf16 datapath (PE 1 cyc/row), f32 PSUM accumulation.
  - One 768-index dma_gather per dst tile (indices int16, wrapped in 16
    partitions and replicated to all eight 16-partition stripes — each
    GPSIMD cpu reads its own stripe). Gathers round-robin over 4 SWDGE
    queues in lock-step with the scheduler's DMASW lane rotation.
  - h tables pre-scaled by dinv^2; self-loop + PSUM drain fused into one
    DVE add (node = pa + hres).
  - One-hot scatter matrices precomputed on the host (norm-scaled bf16)
    and streamed from DRAM per tile — no DVE one-hot builds.
  - Bias via a reserved gather slot per tile: its index points at a bias
    row appended to each h table and its one-hot row is all-ones, so the
    segment-sum matmul adds the layer bias for free.
  - Per-tile PSUM->SBUF stage copy is a single wide activation (ReLU
    folded where the reference has it).
"""
import sys
import time

sys.path.insert(0, "/opt/trn_rl_repo")
import numpy as np
import ml_dtypes
from contextlib import ExitStack

import concourse.bass as bass
import concourse.bacc as bacc
import concourse.mybir as mybir
import concourse.tile as tile
from concourse.bass_utils import run_bass_kernel_spmd
from concourse.masks import make_identity

P = 128
F32 = mybir.dt.float32
BF16 = mybir.dt.bfloat16
I16 = mybir.dt.int16
BF = ml_dtypes.bfloat16


def _pack_graph(src, dst, N):
    """Relabel nodes into degree-balanced 128-node tiles. Every tile
    reserves one gather slot (the 'bias slot'): its index points at the
    bias row (row NP) of the gather table and its one-hot row is all-ones.
    Edge norm is dinv[dst]/dinv[src] (tables store h*dinv^2)."""
    T = (N + P - 1) // P
    NP = T * P
    indeg = np.bincount(dst, minlength=N)          # no-loop in-degree
    C = max(1, int(np.ceil((len(src) + T) / (T * P))))

    order = np.argsort(-indeg, kind="stable")
    while True:
        cap = C * P - 1                             # reserve the bias slot
        load = np.zeros(T, np.int64)
        count = np.zeros(T, np.int64)
        assign = np.empty(N, np.int64)
        ok = True
        for v in order:
            d = int(indeg[v])
            best_t, best_rem = -1, -1
            for t in range(T):
                if count[t] < P:
                    rem = cap - load[t]
                    if rem > best_rem:
                        best_rem, best_t = rem, t
            if best_t < 0 or load[best_t] + d > cap:
                ok = False
                break
            assign[v] = best_t
            load[best_t] += d
            count[best_t] += 1
        if ok:
            break
        C += 1

    perm = np.full(NP, -1, np.int64)
    new_of_old = np.empty(N, np.int64)
    cursor = np.zeros(T, np.int64)
    for v in range(N):
        t = assign[v]
        nid = t * P + cursor[t]
        cursor[t] += 1
        perm[nid] = v
        new_of_old[v] = nid

    # symmetric normalization (degree INCLUDES self-loops, per GCN)
    deg = (indeg + 1).astype(np.float32)
    dinv = (1.0 / np.sqrt(deg, dtype=np.float32)).astype(np.float32)
    norm = (dinv[dst] / dinv[src]).astype(np.float32)

    src_n = new_of_old[src]
    dst_n = new_of_old[dst]
    tile_of_e = dst_n // P
    order_e = np.argsort(tile_of_e, kind="stable")
    src_n, dst_n, norm = src_n[order_e], dst_n[order_e], norm[order_e]
    tile_of_e = tile_of_e[order_e]

    CP = C * P
    gsrc = np.zeros((T, CP), np.int32)
    slot = np.full((T, CP), -1.0, np.float32)       # -1: one-hot row all-zero
    nrm = np.zeros((T, CP), np.float32)
    starts = np.searchsorted(tile_of_e, np.arange(T + 1))
    # bias slot fixed at flat (C-1)*128 (chunk C-1, partition 0 — engines
    # can't address APs starting at partition 127); edges skip that slot
    BSLOT = (C - 1) * P
    pos = np.concatenate([np.arange(BSLOT), np.arange(BSLOT + 1, CP)])
    for t in range(T):
        lo, hi = starts[t], starts[t + 1]
        n_e = hi - lo
        assert n_e <= CP - 1, (t, n_e, CP)
        fs = np.zeros(CP, np.int32)
        fs[pos[:n_e]] = src_n[lo:hi]
        fs[BSLOT] = NP                  # bias slot -> bias row; its all-ones
        gsrc[t] = fs                    # one-hot row is device-built
        slot[t, pos[:n_e]] = (dst_n[lo:hi] - t * P).astype(np.float32)
        nrm[t, pos[:n_e]] = norm[lo:hi]

    # int16 index table: wrapped [i%16, i//16], replicated to all 8 stripes
    SW = CP // 16
    idx16 = np.zeros((P, T * SW), np.int16)
    for t in range(T):
        flat = gsrc[t].astype(np.int16)
        idx16[:, t * SW:(t + 1) * SW] = np.tile(flat.reshape(SW, 16).T, (8, 1))

    dinv_new = np.zeros(NP, np.float32)
    valid = perm >= 0
    dinv_new[valid] = dinv[perm[valid]]
    dinv2 = (dinv_new ** 2).reshape(T, P).T.copy()   # [128, T]

    def dev(a):
        # [T, C, P] -> [P(slot e), T*C] device table
        return np.ascontiguousarray(
            a.reshape(T, C, P).transpose(2, 0, 1).reshape(P, T * C))

    return dict(NP=NP, T=T, C=C, SW=SW, perm=perm,
                dinv2=np.ascontiguousarray(dinv2), idx16=idx16,
                slot=dev(slot).astype(BF), norm=dev(nrm).astype(BF))


def _build_nc(NP, T, C, SW, FM, F5, FO):
    import os
    scratch = int(os.environ.get("KBASS_SCRATCH", "32768"))
    nc = bacc.Bacc("TRN2", dynamic_dma_scratch_size=scratch,
                   num_swdge_queues=4)
    KM = FM // P
    NI = C * P   # gather indices per dst tile (includes the bias slot)
    NR = NP + 1  # gather tables have a bias row at NP

    d = {}
    d["xT1"] = nc.dram_tensor("xT1", [3, NP], BF16, kind="ExternalInput")
    d["hcrow"] = nc.dram_tensor("hcrow", [1, FM], BF16, kind="ExternalInput")
    d["W1v"] = nc.dram_tensor("W1v", [3, FM], BF16, kind="ExternalInput")
    for i in (2, 3, 4):
        d[f"W{i}"] = nc.dram_tensor(f"W{i}", [FM, FM], BF16, kind="ExternalInput")
    d["W5"] = nc.dram_tensor("W5", [FM, F5], BF16, kind="ExternalInput")
    d["W6"] = nc.dram_tensor("W6", [F5, FO], BF16, kind="ExternalInput")
    d["Brows"] = nc.dram_tensor("Brows", [4, FM], BF16, kind="ExternalInput")
    d["Brows56"] = nc.dram_tensor("Brows56", [2, P], BF16, kind="ExternalInput")
    d["b6rep"] = nc.dram_tensor("b6rep", [P, FO], F32, kind="ExternalInput")
    d["idx16"] = nc.dram_tensor("idx16", [P, T * SW], I16, kind="ExternalInput")
    d["slotb"] = nc.dram_tensor("slotb", [P, T * C], BF16, kind="ExternalInput")
    d["normb"] = nc.dram_tensor("normb", [P, T * C], BF16, kind="ExternalInput")
    d["dinv2"] = nc.dram_tensor("dinv2", [P, T], F32, kind="ExternalInput")
    out_d = nc.dram_tensor("out", [NP, FO], F32, kind="ExternalOutput")

    h512 = [nc.dram_tensor(f"h{i}", [NR, FM], BF16, kind="Internal")
            for i in (1, 2, 3, 4)]
    h5t = nc.dram_tensor("h5t", [NR, P], BF16, kind="Internal")
    x6t = nc.dram_tensor("x6t", [NR, P], BF16, kind="Internal")

    Ident = mybir.ActivationFunctionType.Identity
    Relu = mybir.ActivationFunctionType.Relu

    with tile.TileContext(nc) as tc:
        with ExitStack() as ctx:
            res = ctx.enter_context(tc.tile_pool(name="res", bufs=1))
            idx_sb = res.tile([P, T * SW], I16)
            slot_sb = res.tile([P, T * C], BF16)
            norm_sb = res.tile([P, T * C], BF16)
            dinv2_sb = res.tile([P, T], F32)
            b6r_sb = res.tile([P, FO], F32)
            hcrow_sb = res.tile([1, FM], BF16)
            brows_sb = res.tile([4, FM], BF16)
            brows56_sb = res.tile([2, P], BF16)
            for name, t_sb in [("idx16", idx_sb), ("slotb", slot_sb),
                               ("normb", norm_sb), ("dinv2", dinv2_sb),
                               ("b6rep", b6r_sb), ("hcrow", hcrow_sb),
                               ("Brows", brows_sb), ("Brows56", brows56_sb)]:
                nc.sync.dma_start(out=t_sb[:], in_=d[name][:, :])
            reg_ni = nc.gpsimd.to_reg(NI)
            qctr = [0]   # keeps queue_num in lock-step with DMASW lanes
            iota_i = res.tile([P, P], mybir.dt.int32)
            nc.gpsimd.iota(iota_i[:], pattern=[[1, P]], base=0, channel_multiplier=0)
            iota_b = res.tile([P, P], BF16)
            nc.vector.tensor_copy(out=iota_b[:], in_=iota_i[:])
            ident_b = res.tile([P, P], BF16)
            make_identity(nc, ident_b[:])
            hres = [res.tile([P, FM], BF16, name=f"hres_{t}") for t in range(T)]
            hres5 = [res.tile([P, P], BF16, name=f"hres5_{t}") for t in range(T)]
            for t in range(T):
                nc.vector.memset(hres5[t][:, F5:P], 0.0)

            def gather(sp, h_src, t, width, tag):
                g = sp.tile([P, C * width], BF16, tag=tag, name=f"{tag}_{t}")
                nc.gpsimd.dma_gather(
                    out_ap=g[:].rearrange("p (c f) -> p c f", c=C),
                    in_ap=h_src[:, :],
                    idxs_ap=idx_sb[:, t * SW:(t + 1) * SW],
                    num_idxs=NI, num_idxs_reg=reg_ni, elem_size=width,
                    queue_num=qctr[0] % 4)
                qctr[0] += 1
                return g

            def load_oh(sp, t, tag):
                # one-hot built on DVE: (slot == iota) * norm, then the
                # all-ones bias row at the fixed slot (e=127, c=C-1)
                oh = sp.tile([P, NI], BF16, tag=tag, name=f"{tag}_{t}")
                oh3 = oh[:].rearrange("p (c j) -> p c j", c=C)
                nc.vector.tensor_tensor(
                    out=oh3,
                    in0=slot_sb[:, t * C:(t + 1) * C]
                        .rearrange("p (c u) -> p c u", u=1).to_broadcast([P, C, P]),
                    in1=iota_b[:].rearrange("p (u j) -> p u j", u=1)
                        .to_broadcast([P, C, P]),
                    op=mybir.AluOpType.is_equal,
                )
                nc.vector.tensor_tensor(
                    out=oh3, in0=oh3,
                    in1=norm_sb[:, t * C:(t + 1) * C]
                        .rearrange("p (c u) -> p c u", u=1).to_broadcast([P, C, P]),
                    op=mybir.AluOpType.mult,
                )
                nc.vector.memset(oh[0:1, (C - 1) * P:C * P], 1.0)
                return oh

            # ---- layer 1 dense: h1 = (verts @ W1[:3] + img@W1[3:]) * dinv2 ----
            with tc.tile_pool(name="l1", bufs=1) as l1p, \
                 tc.tile_pool(name="l1ps", bufs=2, space="PSUM") as l1ps:
                nc.sync.dma_start(out=h512[0][NP:NP + 1, :], in_=brows_sb[0:1, :])
                xT1_sb = l1p.tile([3, NP], BF16)
                nc.sync.dma_start(out=xT1_sb[:], in_=d["xT1"][:, :])
                W1v_sb = l1p.tile([3, FM], BF16)
                nc.sync.dma_start(out=W1v_sb[:], in_=d["W1v"][:, :])
                ones1 = l1p.tile([1, P], BF16)
                nc.vector.memset(ones1[:], 1.0)
                for t in range(T):
                    pd1 = l1ps.tile([P, FM], F32, tag="pd1", name=f"pd1_{t}")
                    nc.tensor.matmul(out=pd1[:], lhsT=xT1_sb[:, t * P:(t + 1) * P],
                                     rhs=W1v_sb[:], start=True, stop=False)
                    nc.tensor.matmul(out=pd1[:], lhsT=ones1[:], rhs=hcrow_sb[:],
                                     start=False, stop=True)
                    nc.scalar.activation(out=hres[t][:], in_=pd1[:], func=Ident,
                                         scale=dinv2_sb[:, t:t + 1])
                    nc.sync.dma_start(out=h512[0][t * P:(t + 1) * P, :],
                                      in_=hres[t][:])

            # ---- merged phases: scatter(i) + dense(i+1), i = 1..4 ----
            import os as _os
            SBUFS = int(_os.environ.get("KBASS_BUFS", "4"))
            for i in (1, 2, 3, 4):
                relu = i in (2, 4)
                h_src = h512[i - 1]
                F_out = FM if i < 4 else F5
                W_d = d[f"W{i + 1}"]
                with tc.tile_pool(name=f"ph{i}", bufs=SBUFS) as sp, \
                     tc.tile_pool(name=f"ph{i}w", bufs=1) as wp, \
                     tc.tile_pool(name=f"ph{i}ps", bufs=2, space="PSUM") as pp, \
                     tc.tile_pool(name=f"ph{i}pt", bufs=2, space="PSUM") as pt, \
                     tc.tile_pool(name=f"ph{i}pd", bufs=2, space="PSUM") as pd:
                    # bias row of the NEXT phase's gather table
                    if i < 4:
                        nc.sync.dma_start(out=h512[i][NP:NP + 1, :],
                                          in_=brows_sb[i:i + 1, :])
                    else:
                        nc.sync.dma_start(out=h5t[NP:NP + 1, :],
                                          in_=brows56_sb[0:1, :])
                    W_sb = [wp.tile([P, F_out], BF16, tag=f"w{k}", name=f"w{i}_{k}")
                            for k in range(KM)]
                    for k in range(KM):
                        nc.sync.dma_start(out=W_sb[k][:], in_=W_d[k * P:(k + 1) * P, :])
                    for t in range(T):
                        g = gather(sp, h_src, t, FM, f"g{i}")
                        oh = load_oh(sp, t, f"oh{i}")
                        pa = pp.tile([P, FM], F32, tag="pa", name=f"pa{i}_{t}")
                        for c in range(C):
                            nc.tensor.matmul(
                                out=pa[:], lhsT=oh[:, c * P:(c + 1) * P],
                                rhs=g[:, c * FM:(c + 1) * FM],
                                start=(c == 0), stop=(c == C - 1))
                        # self-loop (h rows pre-scaled by dinv^2) fused into
                        # the PSUM->SBUF drain on DVE; bias came via the
                        # all-ones one-hot slot
                        node = sp.tile([P, FM], BF16, tag="node", name=f"nd{i}_{t}")
                        nc.vector.tensor_add(out=node[:], in0=pa[:],
                                             in1=hres[t][:])
                        ptr = pt.tile([P, FM], F32, tag="ptr", name=f"pt{i}_{t}")
                        for fo in range(KM):
                            nc.tensor.matmul(
                                out=ptr[:, fo * P:(fo + 1) * P],
                                lhsT=node[:, fo * P:(fo + 1) * P],
                                rhs=ident_b[:],
                                start=True, stop=True)
                        stage = sp.tile([P, FM], BF16, tag="stage", name=f"st{i}_{t}")
                        nc.scalar.activation(out=stage[:], in_=ptr[:],
                                             func=Relu if relu else Ident)
                        pdn = pd.tile([P, F_out], F32, tag="pdn", name=f"pd{i}_{t}")
                        for k in range(KM):
                            nc.tensor.matmul(out=pdn[:],
                                             lhsT=stage[:, k * P:(k + 1) * P],
                                             rhs=W_sb[k][:], start=(k == 0),
                                             stop=(k == KM - 1))
                        if i < 4:
                            nc.scalar.activation(out=hres[t][:], in_=pdn[:],
                                                 func=Ident,
                                                 scale=dinv2_sb[:, t:t + 1])
                            nc.sync.dma_start(out=h512[i][t * P:(t + 1) * P, :],
                                              in_=hres[t][:])
                        else:
                            nc.scalar.activation(out=hres5[t][:, 0:F5],
                                                 in_=pdn[:], func=Ident,
                                                 scale=dinv2_sb[:, t:t + 1])
                            nc.sync.dma_start(out=h5t[t * P:(t + 1) * P, :],
                                              in_=hres5[t][:])

            # ---- phase 5: x6 = (A_hat h5 + b5) * dinv2, 64-wide ----
            with tc.tile_pool(name="s5", bufs=SBUFS) as sp5, \
                 tc.tile_pool(name="s5ps", bufs=2, space="PSUM") as pp5, \
                 tc.tile_pool(name="s5pt", bufs=2, space="PSUM") as pt5:
                nc.sync.dma_start(out=x6t[NP:NP + 1, :], in_=brows56_sb[1:2, :])
                for t in range(T):
                    g5 = gather(sp5, h5t, t, P, "g5")
                    oh5 = load_oh(sp5, t, "oh5")
                    pg5 = pp5.tile([F5, P], F32, tag="pg5", name=f"pg5_{t}")
                    nc.tensor.matmul(out=pg5[:], lhsT=hres5[t][:, 0:F5],
                                     rhs=ident_b[:],
                                     start=True, stop=False,
                                     skip_group_check=True)
                    for c in range(C):
                        nc.tensor.matmul(out=pg5[:],
                                         lhsT=g5[:, c * P:c * P + F5],
                                         rhs=oh5[:, c * P:(c + 1) * P],
                                         start=False, stop=(c == C - 1),
                                         skip_group_check=True)
                    st6 = sp5.tile([F5, P], BF16, tag="st6", name=f"st6_{t}")
                    nc.scalar.activation(out=st6[:], in_=pg5[:], func=Ident)
                    pt6 = pt5.tile([P, F5], F32, tag="pt6", name=f"pt6_{t}")
                    nc.tensor.matmul(out=pt6[:], lhsT=st6[:],
                                     rhs=ident_b[0:F5, 0:F5],
                                     start=True, stop=True)
                    nc.scalar.activation(out=hres5[t][:, 0:F5], in_=pt6[:],
                                         func=Ident,
                                         scale=dinv2_sb[:, t:t + 1])
                    nc.sync.dma_start(out=x6t[t * P:(t + 1) * P, :],
                                      in_=hres5[t][:])

            # ---- phase 6: out = (A_hat x6) @ W6 + b6 ----
            with tc.tile_pool(name="s6", bufs=SBUFS) as sp6, \
                 tc.tile_pool(name="s6w", bufs=1) as wp6, \
                 tc.tile_pool(name="s6ps", bufs=2, space="PSUM") as pp6, \
                 tc.tile_pool(name="s6pd", bufs=2, space="PSUM") as po6:
                W6_sb = wp6.tile([F5, FO], BF16)
                nc.sync.dma_start(out=W6_sb[:], in_=d["W6"][:, :])
                for t in range(T):
                    g6 = gather(sp6, x6t, t, P, "g6")
                    oh6 = load_oh(sp6, t, "oh6")
                    pg6 = pp6.tile([F5, P], F32, tag="pg6", name=f"pg6_{t}")
                    nc.tensor.matmul(out=pg6[:], lhsT=hres5[t][:, 0:F5],
                                     rhs=ident_b[:],
                                     start=True, stop=False,
                                     skip_group_check=True)
                    for c in range(C):
                        nc.tensor.matmul(out=pg6[:],
                                         lhsT=g6[:, c * P:c * P + F5],
                                         rhs=oh6[:, c * P:(c + 1) * P],
                                         start=False, stop=(c == C - 1),
                                         skip_group_check=True)
                    st7 = sp6.tile([F5, P], BF16, tag="st7", name=f"st7_{t}")
                    nc.scalar.activation(out=st7[:], in_=pg6[:], func=Ident)
                    pout = po6.tile([P, FO], F32, tag="pout", name=f"po_{t}")
                    nc.tensor.matmul(out=pout[:], lhsT=st7[:], rhs=W6_sb[:],
                                     start=True, stop=True)
                    os_ = sp6.tile([P, FO], F32, tag="os", name=f"o_{t}")
                    nc.vector.tensor_add(out=os_[:], in0=pout[:], in1=b6r_sb[:])
                    nc.sync.dma_start(out=out_d[t * P:(t + 1) * P, :], in_=os_[:])

    nc.compile()
    return nc


def _prepare(batch_vertices, img_features, edge_indices,
             W1, b1, W2, b2, W3, b3, W4, b4, W5, b5, W6, b6):
    B, N, _ = batch_vertices.shape
    FM = W1.shape[1]
    F5 = W5.shape[1]
    FO = W6.shape[1]

    ei = np.asarray(edge_indices).astype(np.int64)
    g = _pack_graph(ei[0], ei[1], N)
    NP, T, C, SW, perm = g["NP"], g["T"], g["C"], g["SW"], g["perm"]

    hc = img_features.astype(np.float32) @ W1[3:].astype(np.float32)

    valid = perm >= 0
    vperm = np.zeros((B, NP, 3), np.float32)
    vperm[:, valid, :] = batch_vertices[:, perm[valid], :]

    b5row = np.zeros((1, P), np.float32)
    b5row[0, :F5] = b5
    common = {
        "W1v": np.ascontiguousarray(W1[:3]).astype(BF),
        "W2": np.ascontiguousarray(W2).astype(BF),
        "W3": np.ascontiguousarray(W3).astype(BF),
        "W4": np.ascontiguousarray(W4).astype(BF),
        "W5": np.ascontiguousarray(W5).astype(BF),
        "W6": np.ascontiguousarray(W6).astype(BF),
        "Brows": np.stack([b1, b2, b3, b4]).astype(BF),
        "Brows56": np.concatenate([b5row, np.zeros((1, P), np.float32)]).astype(BF),
        "b6rep": np.tile(b6.astype(np.float32), (P, 1)),
        "idx16": g["idx16"], "slotb": g["slot"], "normb": g["norm"],
        "dinv2": g["dinv2"],
    }
    in_maps = []
    for b in range(B):
        m = dict(common)
        m["xT1"] = np.ascontiguousarray(vperm[b].T).astype(BF)
        m["hcrow"] = hc[b].reshape(1, FM).astype(BF)
        in_maps.append(m)
    meta = dict(NP=NP, T=T, C=C, SW=SW, perm=perm, valid=valid, B=B, N=N,
                FM=FM, F5=F5, FO=FO)
    return in_maps, meta


_BUILD_CACHE = {}


def run(inputs, trace=False):
    in_maps, meta = _prepare(**inputs)
    key = (meta["NP"], meta["C"], meta["FM"], meta["F5"], meta["FO"])
    if key not in _BUILD_CACHE:
        t0 = time.time()
        _BUILD_CACHE[key] = _build_nc(meta["NP"], meta["T"], meta["C"],
                                      meta["SW"], meta["FM"], meta["F5"],
                                      meta["FO"])
        print(f"[kernel] built bass program in {time.time()-t0:.1f}s", file=sys.stderr)
    nc = _BUILD_CACHE[key]
    B = meta["B"]
    res = run_bass_kernel_spmd(nc, in_maps, core_ids=list(range(B)), trace=trace)
    perm, valid, N = meta["perm"], meta["valid"], meta["N"]
    out = np.empty((B, N, meta["FO"]), np.float32)
    for b in range(B):
        dev = res.results[b]["out"]
        out[b, perm[valid], :] = dev[valid, :]
    return out, res


def kernel(**inputs) -> np.ndarray:
    out, _ = run(inputs)
    return out


# revision 22
# speedup vs baseline: 3.0874x; 1.3869x over previous
"""GCN (6-layer GCNConv) Trainium2 Bass kernel — v6.

Data-parallel over batch (1 mesh per NeuronCore). Per layer
out = A_hat @ (x @ W) + b with A_hat = D^-1/2 (A+I) D^-1/2 shared across
batch and layers.

v6 structure (HW: 2.07 ms vs 4.62 ms f32 baseline; rel err 7.8e-3):
  - bf16 datapath (PE 1 cyc/row), f32 PSUM accumulation.
  - One 768-index dma_gather per dst tile (indices int16, wrapped in 16
    partitions and replicated to all eight 16-partition stripes — each
    GPSIMD cpu reads its own stripe). Gathers round-robin over 4 SWDGE
    queues in lock-step with the scheduler's DMASW lane rotation.
  - h tables pre-scaled by dinv^2; self-loop + PSUM drain fused into one
    DVE add (node = pa + hres).
  - One-hot scatter matrices built per tile on the otherwise-idle DVE
    ((slot==iota)*norm, bf16 2x mode); streaming them from DRAM instead
    measured slower (DMA contention stretches the gather critical path).
  - Bias via a reserved gather slot per tile (fixed flat slot (C-1)*128):
    its index points at a bias row appended to each h table and its
    one-hot row is memset to all-ones, so the segment-sum matmul adds the
    layer bias for free.
  - Per-tile PSUM->SBUF stage copy is a single wide activation (ReLU
    folded where the reference has it).
"""
import sys
import time

sys.path.insert(0, "/opt/trn_rl_repo")
import numpy as np
import ml_dtypes
from contextlib import ExitStack

import concourse.bass as bass
import concourse.bacc as bacc
import concourse.mybir as mybir
import concourse.tile as tile
from concourse.bass_utils import run_bass_kernel_spmd
from concourse.masks import make_identity

P = 128
F32 = mybir.dt.float32
BF16 = mybir.dt.bfloat16
I16 = mybir.dt.int16
BF = ml_dtypes.bfloat16


def _pack_graph(src, dst, N):
    """Relabel nodes into degree-balanced 128-node tiles. Every tile
    reserves one gather slot (the 'bias slot'): its index points at the
    bias row (row NP) of the gather table and its one-hot row is all-ones.
    Edge norm is dinv[dst]/dinv[src] (tables store h*dinv^2)."""
    T = (N + P - 1) // P
    NP = T * P
    indeg = np.bincount(dst, minlength=N)          # no-loop in-degree
    C = max(1, int(np.ceil((len(src) + T) / (T * P))))

    order = np.argsort(-indeg, kind="stable")
    while True:
        cap = C * P - 1                             # reserve the bias slot
        load = np.zeros(T, np.int64)
        count = np.zeros(T, np.int64)
        assign = np.empty(N, np.int64)
        ok = True
        for v in order:
            d = int(indeg[v])
            best_t, best_rem = -1, -1
            for t in range(T):
                if count[t] < P:
                    rem = cap - load[t]
                    if rem > best_rem:
                        best_rem, best_t = rem, t
            if best_t < 0 or load[best_t] + d > cap:
                ok = False
                break
            assign[v] = best_t
            load[best_t] += d
            count[best_t] += 1
        if ok:
            break
        C += 1

    perm = np.full(NP, -1, np.int64)
    new_of_old = np.empty(N, np.int64)
    cursor = np.zeros(T, np.int64)
    for v in range(N):
        t = assign[v]
        nid = t * P + cursor[t]
        cursor[t] += 1
        perm[nid] = v
        new_of_old[v] = nid

    # symmetric normalization (degree INCLUDES self-loops, per GCN)
    deg = (indeg + 1).astype(np.float32)
    dinv = (1.0 / np.sqrt(deg, dtype=np.float32)).astype(np.float32)
    norm = (dinv[dst] / dinv[src]).astype(np.float32)

    src_n = new_of_old[src]
    dst_n = new_of_old[dst]
    tile_of_e = dst_n // P
    order_e = np.argsort(tile_of_e, kind="stable")
    src_n, dst_n, norm = src_n[order_e], dst_n[order_e], norm[order_e]
    tile_of_e = tile_of_e[order_e]

    CP = C * P
    gsrc = np.zeros((T, CP), np.int32)
    slot = np.full((T, CP), -1.0, np.float32)       # -1: one-hot row all-zero
    nrm = np.zeros((T, CP), np.float32)
    starts = np.searchsorted(tile_of_e, np.arange(T + 1))
    # bias slot fixed at flat (C-1)*128 (chunk C-1, partition 0 — engines
    # can't address APs starting at partition 127); edges skip that slot
    BSLOT = (C - 1) * P
    pos = np.concatenate([np.arange(BSLOT), np.arange(BSLOT + 1, CP)])
    for t in range(T):
        lo, hi = starts[t], starts[t + 1]
        n_e = hi - lo
        assert n_e <= CP - 1, (t, n_e, CP)
        fs = np.zeros(CP, np.int32)
        fs[pos[:n_e]] = src_n[lo:hi]
        fs[BSLOT] = NP                  # bias slot -> bias row; its all-ones
        gsrc[t] = fs                    # one-hot row is device-built
        slot[t, pos[:n_e]] = (dst_n[lo:hi] - t * P).astype(np.float32)
        nrm[t, pos[:n_e]] = norm[lo:hi]

    # int16 index table: wrapped [i%16, i//16], replicated to all 8 stripes
    SW = CP // 16
    idx16 = np.zeros((P, T * SW), np.int16)
    for t in range(T):
        flat = gsrc[t].astype(np.int16)
        idx16[:, t * SW:(t + 1) * SW] = np.tile(flat.reshape(SW, 16).T, (8, 1))

    dinv_new = np.zeros(NP, np.float32)
    valid = perm >= 0
    dinv_new[valid] = dinv[perm[valid]]
    dinv2 = (dinv_new ** 2).reshape(T, P).T.copy()   # [128, T]

    def dev(a):
        # [T, C, P] -> [P(slot e), T*C] device table
        return np.ascontiguousarray(
            a.reshape(T, C, P).transpose(2, 0, 1).reshape(P, T * C))

    return dict(NP=NP, T=T, C=C, SW=SW, perm=perm,
                dinv2=np.ascontiguousarray(dinv2), idx16=idx16,
                slot=dev(slot).astype(BF), norm=dev(nrm).astype(BF))


def _build_nc(NP, T, C, SW, FM, F5, FO):
    import os
    scratch = int(os.environ.get("KBASS_SCRATCH", "32768"))
    nc = bacc.Bacc("TRN2", dynamic_dma_scratch_size=scratch,
                   num_swdge_queues=4)
    KM = FM // P
    NI = C * P   # gather indices per dst tile (includes the bias slot)
    NR = NP + 1  # gather tables have a bias row at NP

    d = {}
    d["xT1"] = nc.dram_tensor("xT1", [3, NP], BF16, kind="ExternalInput")
    d["hcrow"] = nc.dram_tensor("hcrow", [1, FM], BF16, kind="ExternalInput")
    d["W1v"] = nc.dram_tensor("W1v", [3, FM], BF16, kind="ExternalInput")
    for i in (2, 3, 4):
        d[f"W{i}"] = nc.dram_tensor(f"W{i}", [FM, FM], BF16, kind="ExternalInput")
    d["W5"] = nc.dram_tensor("W5", [FM, F5], BF16, kind="ExternalInput")
    d["W6"] = nc.dram_tensor("W6", [F5, FO], BF16, kind="ExternalInput")
    d["Brows"] = nc.dram_tensor("Brows", [4, FM], BF16, kind="ExternalInput")
    d["Brows56"] = nc.dram_tensor("Brows56", [2, P], BF16, kind="ExternalInput")
    d["b6rep"] = nc.dram_tensor("b6rep", [P, FO], F32, kind="ExternalInput")
    d["idx16"] = nc.dram_tensor("idx16", [P, T * SW], I16, kind="ExternalInput")
    d["slotb"] = nc.dram_tensor("slotb", [P, T * C], BF16, kind="ExternalInput")
    d["normb"] = nc.dram_tensor("normb", [P, T * C], BF16, kind="ExternalInput")
    d["dinv2"] = nc.dram_tensor("dinv2", [P, T], F32, kind="ExternalInput")
    out_d = nc.dram_tensor("out", [NP, FO], F32, kind="ExternalOutput")

    h512 = [nc.dram_tensor(f"h{i}", [NR, FM], BF16, kind="Internal")
            for i in (1, 2, 3, 4)]
    h5t = nc.dram_tensor("h5t", [NR, P], BF16, kind="Internal")
    x6t = nc.dram_tensor("x6t", [NR, P], BF16, kind="Internal")

    Ident = mybir.ActivationFunctionType.Identity
    Relu = mybir.ActivationFunctionType.Relu

    with tile.TileContext(nc) as tc:
        with ExitStack() as ctx:
            res = ctx.enter_context(tc.tile_pool(name="res", bufs=1))
            idx_sb = res.tile([P, T * SW], I16)
            slot_sb = res.tile([P, T * C], BF16)
            norm_sb = res.tile([P, T * C], BF16)
            dinv2_sb = res.tile([P, T], F32)
            b6r_sb = res.tile([P, FO], F32)
            hcrow_sb = res.tile([1, FM], BF16)
            brows_sb = res.tile([4, FM], BF16)
            brows56_sb = res.tile([2, P], BF16)
            for name, t_sb in [("idx16", idx_sb), ("slotb", slot_sb),
                               ("normb", norm_sb), ("dinv2", dinv2_sb),
                               ("b6rep", b6r_sb), ("hcrow", hcrow_sb),
                               ("Brows", brows_sb), ("Brows56", brows56_sb)]:
                nc.sync.dma_start(out=t_sb[:], in_=d[name][:, :])
            reg_ni = nc.gpsimd.to_reg(NI)
            qctr = [0]   # keeps queue_num in lock-step with DMASW lanes
            iota_i = res.tile([P, P], mybir.dt.int32)
            nc.gpsimd.iota(iota_i[:], pattern=[[1, P]], base=0, channel_multiplier=0)
            iota_b = res.tile([P, P], BF16)
            nc.vector.tensor_copy(out=iota_b[:], in_=iota_i[:])
            ident_b = res.tile([P, P], BF16)
            make_identity(nc, ident_b[:])
            hres = [res.tile([P, FM], BF16, name=f"hres_{t}") for t in range(T)]
            hres5 = [res.tile([P, P], BF16, name=f"hres5_{t}") for t in range(T)]
            for t in range(T):
                nc.vector.memset(hres5[t][:, F5:P], 0.0)

            def gather(sp, h_src, t, width, tag):
                g = sp.tile([P, C * width], BF16, tag=tag, name=f"{tag}_{t}")
                nc.gpsimd.dma_gather(
                    out_ap=g[:].rearrange("p (c f) -> p c f", c=C),
                    in_ap=h_src[:, :],
                    idxs_ap=idx_sb[:, t * SW:(t + 1) * SW],
                    num_idxs=NI, num_idxs_reg=reg_ni, elem_size=width,
                    queue_num=qctr[0] % 4)
                qctr[0] += 1
                return g

            def load_oh(sp, t, tag):
                # one-hot built on DVE: (slot == iota) * norm, then the
                # all-ones bias row at the fixed slot (e=127, c=C-1)
                oh = sp.tile([P, NI], BF16, tag=tag, name=f"{tag}_{t}")
                oh3 = oh[:].rearrange("p (c j) -> p c j", c=C)
                nc.vector.tensor_tensor(
                    out=oh3,
                    in0=slot_sb[:, t * C:(t + 1) * C]
                        .rearrange("p (c u) -> p c u", u=1).to_broadcast([P, C, P]),
                    in1=iota_b[:].rearrange("p (u j) -> p u j", u=1)
                        .to_broadcast([P, C, P]),
                    op=mybir.AluOpType.is_equal,
                )
                nc.vector.tensor_tensor(
                    out=oh3, in0=oh3,
                    in1=norm_sb[:, t * C:(t + 1) * C]
                        .rearrange("p (c u) -> p c u", u=1).to_broadcast([P, C, P]),
                    op=mybir.AluOpType.mult,
                )
                nc.vector.memset(oh[0:1, (C - 1) * P:C * P], 1.0)
                return oh

            # ---- layer 1 dense: h1 = (verts @ W1[:3] + img@W1[3:]) * dinv2 ----
            with tc.tile_pool(name="l1", bufs=1) as l1p, \
                 tc.tile_pool(name="l1ps", bufs=2, space="PSUM") as l1ps:
                nc.sync.dma_start(out=h512[0][NP:NP + 1, :], in_=brows_sb[0:1, :])
                xT1_sb = l1p.tile([3, NP], BF16)
                nc.sync.dma_start(out=xT1_sb[:], in_=d["xT1"][:, :])
                W1v_sb = l1p.tile([3, FM], BF16)
                nc.sync.dma_start(out=W1v_sb[:], in_=d["W1v"][:, :])
                ones1 = l1p.tile([1, P], BF16)
                nc.vector.memset(ones1[:], 1.0)
                for t in range(T):
                    pd1 = l1ps.tile([P, FM], F32, tag="pd1", name=f"pd1_{t}")
                    nc.tensor.matmul(out=pd1[:], lhsT=xT1_sb[:, t * P:(t + 1) * P],
                                     rhs=W1v_sb[:], start=True, stop=False)
                    nc.tensor.matmul(out=pd1[:], lhsT=ones1[:], rhs=hcrow_sb[:],
                                     start=False, stop=True)
                    nc.scalar.activation(out=hres[t][:], in_=pd1[:], func=Ident,
                                         scale=dinv2_sb[:, t:t + 1])
                    nc.sync.dma_start(out=h512[0][t * P:(t + 1) * P, :],
                                      in_=hres[t][:])

            # ---- merged phases: scatter(i) + dense(i+1), i = 1..4 ----
            import os as _os
            SBUFS = int(_os.environ.get("KBASS_BUFS", "6"))
            for i in (1, 2, 3, 4):
                relu = i in (2, 4)
                h_src = h512[i - 1]
                F_out = FM if i < 4 else F5
                W_d = d[f"W{i + 1}"]
                with tc.tile_pool(name=f"ph{i}", bufs=SBUFS) as sp, \
                     tc.tile_pool(name=f"ph{i}w", bufs=1) as wp, \
                     tc.tile_pool(name=f"ph{i}ps", bufs=3, space="PSUM") as pp, \
                     tc.tile_pool(name=f"ph{i}pt", bufs=2, space="PSUM") as pt, \
                     tc.tile_pool(name=f"ph{i}pd", bufs=2, space="PSUM") as pd:
                    # bias row of the NEXT phase's gather table
                    if i < 4:
                        nc.sync.dma_start(out=h512[i][NP:NP + 1, :],
                                          in_=brows_sb[i:i + 1, :])
                    else:
                        nc.sync.dma_start(out=h5t[NP:NP + 1, :],
                                          in_=brows56_sb[0:1, :])
                    W_sb = [wp.tile([P, F_out], BF16, tag=f"w{k}", name=f"w{i}_{k}")
                            for k in range(KM)]
                    for k in range(KM):
                        nc.sync.dma_start(out=W_sb[k][:], in_=W_d[k * P:(k + 1) * P, :])
                    for t in range(T):
                        g = gather(sp, h_src, t, FM, f"g{i}")
                        oh = load_oh(sp, t, f"oh{i}")
                        pa = pp.tile([P, FM], F32, tag="pa", name=f"pa{i}_{t}")
                        for c in range(C):
                            nc.tensor.matmul(
                                out=pa[:], lhsT=oh[:, c * P:(c + 1) * P],
                                rhs=g[:, c * FM:(c + 1) * FM],
                                start=(c == 0), stop=(c == C - 1))
                        # self-loop (h rows pre-scaled by dinv^2) fused into
                        # the PSUM->SBUF drain on DVE; bias came via the
                        # all-ones one-hot slot
                        node = sp.tile([P, FM], BF16, tag="node", name=f"nd{i}_{t}")
                        nc.vector.tensor_add(out=node[:], in0=pa[:],
                                             in1=hres[t][:])
                        ptr = pt.tile([P, FM], F32, tag="ptr", name=f"pt{i}_{t}")
                        for fo in range(KM):
                            nc.tensor.matmul(
                                out=ptr[:, fo * P:(fo + 1) * P],
                                lhsT=node[:, fo * P:(fo + 1) * P],
                                rhs=ident_b[:],
                                start=True, stop=True)
                        stage = sp.tile([P, FM], BF16, tag="stage", name=f"st{i}_{t}")
                        nc.scalar.activation(out=stage[:], in_=ptr[:],
                                             func=Relu if relu else Ident)
                        pdn = pd.tile([P, F_out], F32, tag="pdn", name=f"pd{i}_{t}")
                        for k in range(KM):
                            nc.tensor.matmul(out=pdn[:],
                                             lhsT=stage[:, k * P:(k + 1) * P],
                                             rhs=W_sb[k][:], start=(k == 0),
                                             stop=(k == KM - 1))
                        if i < 4:
                            nc.scalar.activation(out=hres[t][:], in_=pdn[:],
                                                 func=Ident,
                                                 scale=dinv2_sb[:, t:t + 1])
                            nc.sync.dma_start(out=h512[i][t * P:(t + 1) * P, :],
                                              in_=hres[t][:])
                        else:
                            nc.scalar.activation(out=hres5[t][:, 0:F5],
                                                 in_=pdn[:], func=Ident,
                                                 scale=dinv2_sb[:, t:t + 1])
                            nc.sync.dma_start(out=h5t[t * P:(t + 1) * P, :],
                                              in_=hres5[t][:])

            # ---- phase 5: x6 = (A_hat h5 + b5) * dinv2, 64-wide ----
            with tc.tile_pool(name="s5", bufs=SBUFS) as sp5, \
                 tc.tile_pool(name="s5ps", bufs=2, space="PSUM") as pp5, \
                 tc.tile_pool(name="s5pt", bufs=2, space="PSUM") as pt5:
                nc.sync.dma_start(out=x6t[NP:NP + 1, :], in_=brows56_sb[1:2, :])
                for t in range(T):
                    g5 = gather(sp5, h5t, t, P, "g5")
                    oh5 = load_oh(sp5, t, "oh5")
                    pg5 = pp5.tile([F5, P], F32, tag="pg5", name=f"pg5_{t}")
                    nc.tensor.matmul(out=pg5[:], lhsT=hres5[t][:, 0:F5],
                                     rhs=ident_b[:],
                                     start=True, stop=False,
                                     skip_group_check=True)
                    for c in range(C):
                        nc.tensor.matmul(out=pg5[:],
                                         lhsT=g5[:, c * P:c * P + F5],
                                         rhs=oh5[:, c * P:(c + 1) * P],
                                         start=False, stop=(c == C - 1),
                                         skip_group_check=True)
                    st6 = sp5.tile([F5, P], BF16, tag="st6", name=f"st6_{t}")
                    nc.scalar.activation(out=st6[:], in_=pg5[:], func=Ident)
                    pt6 = pt5.tile([P, F5], F32, tag="pt6", name=f"pt6_{t}")
                    nc.tensor.matmul(out=pt6[:], lhsT=st6[:],
                                     rhs=ident_b[0:F5, 0:F5],
                                     start=True, stop=True)
                    nc.scalar.activation(out=hres5[t][:, 0:F5], in_=pt6[:],
                                         func=Ident,
                                         scale=dinv2_sb[:, t:t + 1])
                    nc.sync.dma_start(out=x6t[t * P:(t + 1) * P, :],
                                      in_=hres5[t][:])

            # ---- phase 6: out = (A_hat x6) @ W6 + b6 ----
            with tc.tile_pool(name="s6", bufs=SBUFS) as sp6, \
                 tc.tile_pool(name="s6w", bufs=1) as wp6, \
                 tc.tile_pool(name="s6ps", bufs=2, space="PSUM") as pp6, \
                 tc.tile_pool(name="s6pd", bufs=2, space="PSUM") as po6:
                W6_sb = wp6.tile([F5, FO], BF16)
                nc.sync.dma_start(out=W6_sb[:], in_=d["W6"][:, :])
                for t in range(T):
                    g6 = gather(sp6, x6t, t, P, "g6")
                    oh6 = load_oh(sp6, t, "oh6")
                    pg6 = pp6.tile([F5, P], F32, tag="pg6", name=f"pg6_{t}")
                    nc.tensor.matmul(out=pg6[:], lhsT=hres5[t][:, 0:F5],
                                     rhs=ident_b[:],
                                     start=True, stop=False,
                                     skip_group_check=True)
                    for c in range(C):
                        nc.tensor.matmul(out=pg6[:],
                                         lhsT=g6[:, c * P:c * P + F5],
                                         rhs=oh6[:, c * P:(c + 1) * P],
                                         start=False, stop=(c == C - 1),
                                         skip_group_check=True)
                    st7 = sp6.tile([F5, P], BF16, tag="st7", name=f"st7_{t}")
                    nc.scalar.activation(out=st7[:], in_=pg6[:], func=Ident)
                    pout = po6.tile([P, FO], F32, tag="pout", name=f"po_{t}")
                    nc.tensor.matmul(out=pout[:], lhsT=st7[:], rhs=W6_sb[:],
                                     start=True, stop=True)
                    os_ = sp6.tile([P, FO], F32, tag="os", name=f"o_{t}")
                    nc.vector.tensor_add(out=os_[:], in0=pout[:], in1=b6r_sb[:])
                    nc.sync.dma_start(out=out_d[t * P:(t + 1) * P, :], in_=os_[:])

    nc.compile()
    return nc


def _prepare(batch_vertices, img_features, edge_indices,
             W1, b1, W2, b2, W3, b3, W4, b4, W5, b5, W6, b6):
    B, N, _ = batch_vertices.shape
    FM = W1.shape[1]
    F5 = W5.shape[1]
    FO = W6.shape[1]

    ei = np.asarray(edge_indices).astype(np.int64)
    g = _pack_graph(ei[0], ei[1], N)
    NP, T, C, SW, perm = g["NP"], g["T"], g["C"], g["SW"], g["perm"]

    hc = img_features.astype(np.float32) @ W1[3:].astype(np.float32)

    valid = perm >= 0
    vperm = np.zeros((B, NP, 3), np.float32)
    vperm[:, valid, :] = batch_vertices[:, perm[valid], :]

    b5row = np.zeros((1, P), np.float32)
    b5row[0, :F5] = b5
    common = {
        "W1v": np.ascontiguousarray(W1[:3]).astype(BF),
        "W2": np.ascontiguousarray(W2).astype(BF),
        "W3": np.ascontiguousarray(W3).astype(BF),
        "W4": np.ascontiguousarray(W4).astype(BF),
        "W5": np.ascontiguousarray(W5).astype(BF),
        "W6": np.ascontiguousarray(W6).astype(BF),
        "Brows": np.stack([b1, b2, b3, b4]).astype(BF),
        "Brows56": np.concatenate([b5row, np.zeros((1, P), np.float32)]).astype(BF),
        "b6rep": np.tile(b6.astype(np.float32), (P, 1)),
        "idx16": g["idx16"], "slotb": g["slot"], "normb": g["norm"],
        "dinv2": g["dinv2"],
    }
    in_maps = []
    for b in range(B):
        m = dict(common)
        m["xT1"] = np.ascontiguousarray(vperm[b].T).astype(BF)
        m["hcrow"] = hc[b].reshape(1, FM).astype(BF)
        in_maps.append(m)
    meta = dict(NP=NP, T=T, C=C, SW=SW, perm=perm, valid=valid, B=B, N=N,
                FM=FM, F5=F5, FO=FO)
    return in_maps, meta


_BUILD_CACHE = {}


def run(inputs, trace=False):
    in_maps, meta = _prepare(**inputs)
    key = (meta["NP"], meta["C"], meta["FM"], meta["F5"], meta["FO"])
    if key not in _BUILD_CACHE:
        t0 = time.time()
        _BUILD_CACHE[key] = _build_nc(meta["NP"], meta["T"], meta["C"],
                                      meta["SW"], meta["FM"], meta["F5"],
                                      meta["FO"])
        print(f"[kernel] built bass program in {time.time()-t0:.1f}s", file=sys.stderr)
    nc = _BUILD_CACHE[key]
    B = meta["B"]
    res = run_bass_kernel_spmd(nc, in_maps, core_ids=list(range(B)), trace=trace)
    perm, valid, N = meta["perm"], meta["valid"], meta["N"]
    out = np.empty((B, N, meta["FO"]), np.float32)
    for b in range(B):
        dev = res.results[b]["out"]
        out[b, perm[valid], :] = dev[valid, :]
    return out, res


def kernel(**inputs) -> np.ndarray:
    out, _ = run(inputs)
    return out


# revision 24
# speedup vs baseline: 3.1677x; 1.0260x over previous
"""GCN (6-layer GCNConv) Trainium2 Bass kernel — v6.

Data-parallel over batch (1 mesh per NeuronCore). Per layer
out = A_hat @ (x @ W) + b with A_hat = D^-1/2 (A+I) D^-1/2 shared across
batch and layers.

v6 structure (HW: 1.49 ms vs 4.62 ms f32 baseline; rel err 7.8e-3;
KBASS_BUFS=6 + 3 PSUM scatter banks let the 4 SWDGE queue pairs actually
run concurrently):
  - bf16 datapath (PE 1 cyc/row), f32 PSUM accumulation.
  - One 768-index dma_gather per dst tile (indices int16, wrapped in 16
    partitions and replicated to all eight 16-partition stripes — each
    GPSIMD cpu reads its own stripe). Gathers round-robin over 4 SWDGE
    queues in lock-step with the scheduler's DMASW lane rotation.
  - h tables pre-scaled by dinv^2; self-loop + PSUM drain fused into one
    DVE add (node = pa + hres).
  - One-hot scatter matrices built per tile on the otherwise-idle DVE
    ((slot==iota)*norm, bf16 2x mode); streaming them from DRAM instead
    measured slower (DMA contention stretches the gather critical path).
  - Bias via a reserved gather slot per tile (fixed flat slot (C-1)*128):
    its index points at a bias row appended to each h table and its
    one-hot row is memset to all-ones, so the segment-sum matmul adds the
    layer bias for free.
  - Per-tile PSUM->SBUF stage copy is a single wide activation (ReLU
    folded where the reference has it).
"""
import sys
import time

sys.path.insert(0, "/opt/trn_rl_repo")
import numpy as np
import ml_dtypes
from contextlib import ExitStack

import concourse.bass as bass
import concourse.bacc as bacc
import concourse.mybir as mybir
import concourse.tile as tile
from concourse.bass_utils import run_bass_kernel_spmd
from concourse.masks import make_identity

P = 128
F32 = mybir.dt.float32
BF16 = mybir.dt.bfloat16
I16 = mybir.dt.int16
BF = ml_dtypes.bfloat16


def _pack_graph(src, dst, N):
    """Relabel nodes into degree-balanced 128-node tiles. Every tile
    reserves one gather slot (the 'bias slot'): its index points at the
    bias row (row NP) of the gather table and its one-hot row is all-ones.
    Edge norm is dinv[dst]/dinv[src] (tables store h*dinv^2)."""
    T = (N + P - 1) // P
    NP = T * P
    indeg = np.bincount(dst, minlength=N)          # no-loop in-degree
    C = max(1, int(np.ceil((len(src) + T) / (T * P))))

    order = np.argsort(-indeg, kind="stable")
    while True:
        cap = C * P - 1                             # reserve the bias slot
        load = np.zeros(T, np.int64)
        count = np.zeros(T, np.int64)
        assign = np.empty(N, np.int64)
        ok = True
        for v in order:
            d = int(indeg[v])
            best_t, best_rem = -1, -1
            for t in range(T):
                if count[t] < P:
                    rem = cap - load[t]
                    if rem > best_rem:
                        best_rem, best_t = rem, t
            if best_t < 0 or load[best_t] + d > cap:
                ok = False
                break
            assign[v] = best_t
            load[best_t] += d
            count[best_t] += 1
        if ok:
            break
        C += 1

    perm = np.full(NP, -1, np.int64)
    new_of_old = np.empty(N, np.int64)
    cursor = np.zeros(T, np.int64)
    for v in range(N):
        t = assign[v]
        nid = t * P + cursor[t]
        cursor[t] += 1
        perm[nid] = v
        new_of_old[v] = nid

    # symmetric normalization (degree INCLUDES self-loops, per GCN)
    deg = (indeg + 1).astype(np.float32)
    dinv = (1.0 / np.sqrt(deg, dtype=np.float32)).astype(np.float32)
    norm = (dinv[dst] / dinv[src]).astype(np.float32)

    src_n = new_of_old[src]
    dst_n = new_of_old[dst]
    tile_of_e = dst_n // P
    order_e = np.argsort(tile_of_e, kind="stable")
    src_n, dst_n, norm = src_n[order_e], dst_n[order_e], norm[order_e]
    tile_of_e = tile_of_e[order_e]

    CP = C * P
    gsrc = np.zeros((T, CP), np.int32)
    slot = np.full((T, CP), -1.0, np.float32)       # -1: one-hot row all-zero
    nrm = np.zeros((T, CP), np.float32)
    starts = np.searchsorted(tile_of_e, np.arange(T + 1))
    # bias slot fixed at flat (C-1)*128 (chunk C-1, partition 0 — engines
    # can't address APs starting at partition 127); edges skip that slot
    BSLOT = (C - 1) * P
    pos = np.concatenate([np.arange(BSLOT), np.arange(BSLOT + 1, CP)])
    for t in range(T):
        lo, hi = starts[t], starts[t + 1]
        n_e = hi - lo
        assert n_e <= CP - 1, (t, n_e, CP)
        fs = np.zeros(CP, np.int32)
        fs[pos[:n_e]] = src_n[lo:hi]
        fs[BSLOT] = NP                  # bias slot -> bias row; its all-ones
        gsrc[t] = fs                    # one-hot row is device-built
        slot[t, pos[:n_e]] = (dst_n[lo:hi] - t * P).astype(np.float32)
        nrm[t, pos[:n_e]] = norm[lo:hi]

    # int16 index table: wrapped [i%16, i//16], replicated to all 8 stripes
    SW = CP // 16
    idx16 = np.zeros((P, T * SW), np.int16)
    for t in range(T):
        flat = gsrc[t].astype(np.int16)
        idx16[:, t * SW:(t + 1) * SW] = np.tile(flat.reshape(SW, 16).T, (8, 1))

    dinv_new = np.zeros(NP, np.float32)
    valid = perm >= 0
    dinv_new[valid] = dinv[perm[valid]]
    dinv2 = (dinv_new ** 2).reshape(T, P).T.copy()   # [128, T]

    def dev(a):
        # [T, C, P] -> [P(slot e), T*C] device table
        return np.ascontiguousarray(
            a.reshape(T, C, P).transpose(2, 0, 1).reshape(P, T * C))

    return dict(NP=NP, T=T, C=C, SW=SW, perm=perm,
                dinv2=np.ascontiguousarray(dinv2), idx16=idx16,
                slot=dev(slot).astype(BF), norm=dev(nrm).astype(BF))


def _build_nc(NP, T, C, SW, FM, F5, FO):
    import os
    scratch = int(os.environ.get("KBASS_SCRATCH", "32768"))
    nc = bacc.Bacc("TRN2", dynamic_dma_scratch_size=scratch,
                   num_swdge_queues=4)
    KM = FM // P
    NI = C * P   # gather indices per dst tile (includes the bias slot)
    NR = NP + 1  # gather tables have a bias row at NP

    d = {}
    d["xT1"] = nc.dram_tensor("xT1", [3, NP], BF16, kind="ExternalInput")
    d["hcrow"] = nc.dram_tensor("hcrow", [1, FM], BF16, kind="ExternalInput")
    d["W1v"] = nc.dram_tensor("W1v", [3, FM], BF16, kind="ExternalInput")
    for i in (2, 3, 4):
        d[f"W{i}"] = nc.dram_tensor(f"W{i}", [FM, FM], BF16, kind="ExternalInput")
    d["W5"] = nc.dram_tensor("W5", [FM, F5], BF16, kind="ExternalInput")
    d["W6"] = nc.dram_tensor("W6", [F5, FO], BF16, kind="ExternalInput")
    d["Brows"] = nc.dram_tensor("Brows", [4, FM], BF16, kind="ExternalInput")
    d["Brows56"] = nc.dram_tensor("Brows56", [2, P], BF16, kind="ExternalInput")
    d["b6rep"] = nc.dram_tensor("b6rep", [P, FO], F32, kind="ExternalInput")
    d["idx16"] = nc.dram_tensor("idx16", [P, T * SW], I16, kind="ExternalInput")
    d["slotb"] = nc.dram_tensor("slotb", [P, T * C], BF16, kind="ExternalInput")
    d["normb"] = nc.dram_tensor("normb", [P, T * C], BF16, kind="ExternalInput")
    d["dinv2"] = nc.dram_tensor("dinv2", [P, T], F32, kind="ExternalInput")
    out_d = nc.dram_tensor("out", [NP, FO], F32, kind="ExternalOutput")

    h512 = [nc.dram_tensor(f"h{i}", [NR, FM], BF16, kind="Internal")
            for i in (1, 2, 3, 4)]
    h5t = nc.dram_tensor("h5t", [NR, P], BF16, kind="Internal")
    x6t = nc.dram_tensor("x6t", [NR, P], BF16, kind="Internal")

    Ident = mybir.ActivationFunctionType.Identity
    Relu = mybir.ActivationFunctionType.Relu

    with tile.TileContext(nc) as tc:
        with ExitStack() as ctx:
            res = ctx.enter_context(tc.tile_pool(name="res", bufs=1))
            idx_sb = res.tile([P, T * SW], I16)
            slot_sb = res.tile([P, T * C], BF16)
            norm_sb = res.tile([P, T * C], BF16)
            dinv2_sb = res.tile([P, T], F32)
            b6r_sb = res.tile([P, FO], F32)
            hcrow_sb = res.tile([1, FM], BF16)
            brows_sb = res.tile([4, FM], BF16)
            brows56_sb = res.tile([2, P], BF16)
            for name, t_sb in [("idx16", idx_sb), ("slotb", slot_sb),
                               ("normb", norm_sb), ("dinv2", dinv2_sb),
                               ("b6rep", b6r_sb), ("hcrow", hcrow_sb),
                               ("Brows", brows_sb), ("Brows56", brows56_sb)]:
                nc.sync.dma_start(out=t_sb[:], in_=d[name][:, :])
            reg_ni = nc.gpsimd.to_reg(NI)
            qctr = [0]   # keeps queue_num in lock-step with DMASW lanes
            iota_i = res.tile([P, P], mybir.dt.int32)
            nc.gpsimd.iota(iota_i[:], pattern=[[1, P]], base=0, channel_multiplier=0)
            iota_b = res.tile([P, P], BF16)
            nc.vector.tensor_copy(out=iota_b[:], in_=iota_i[:])
            ident_b = res.tile([P, P], BF16)
            make_identity(nc, ident_b[:])
            hres = [res.tile([P, FM], BF16, name=f"hres_{t}") for t in range(T)]
            hres5 = [res.tile([P, P], BF16, name=f"hres5_{t}") for t in range(T)]
            for t in range(T):
                nc.vector.memset(hres5[t][:, F5:P], 0.0)

            def gather(sp, h_src, t, width, tag):
                g = sp.tile([P, C * width], BF16, tag=tag, name=f"{tag}_{t}")
                nc.gpsimd.dma_gather(
                    out_ap=g[:].rearrange("p (c f) -> p c f", c=C),
                    in_ap=h_src[:, :],
                    idxs_ap=idx_sb[:, t * SW:(t + 1) * SW],
                    num_idxs=NI, num_idxs_reg=reg_ni, elem_size=width,
                    queue_num=qctr[0] % 4)
                qctr[0] += 1
                return g

            def load_oh(sp, t, tag):
                # one-hot built on DVE: (slot == iota) * norm, then the
                # all-ones bias row at the fixed slot (e=127, c=C-1)
                oh = sp.tile([P, NI], BF16, tag=tag, name=f"{tag}_{t}")
                oh3 = oh[:].rearrange("p (c j) -> p c j", c=C)
                nc.vector.tensor_tensor(
                    out=oh3,
                    in0=slot_sb[:, t * C:(t + 1) * C]
                        .rearrange("p (c u) -> p c u", u=1).to_broadcast([P, C, P]),
                    in1=iota_b[:].rearrange("p (u j) -> p u j", u=1)
                        .to_broadcast([P, C, P]),
                    op=mybir.AluOpType.is_equal,
                )
                nc.vector.tensor_tensor(
                    out=oh3, in0=oh3,
                    in1=norm_sb[:, t * C:(t + 1) * C]
                        .rearrange("p (c u) -> p c u", u=1).to_broadcast([P, C, P]),
                    op=mybir.AluOpType.mult,
                )
                nc.vector.memset(oh[0:1, (C - 1) * P:C * P], 1.0)
                return oh

            # ---- layer 1 dense: h1 = (verts @ W1[:3] + img@W1[3:]) * dinv2 ----
            with tc.tile_pool(name="l1", bufs=1) as l1p, \
                 tc.tile_pool(name="l1ps", bufs=2, space="PSUM") as l1ps:
                nc.sync.dma_start(out=h512[0][NP:NP + 1, :], in_=brows_sb[0:1, :])
                xT1_sb = l1p.tile([3, NP], BF16)
                nc.sync.dma_start(out=xT1_sb[:], in_=d["xT1"][:, :])
                W1v_sb = l1p.tile([3, FM], BF16)
                nc.sync.dma_start(out=W1v_sb[:], in_=d["W1v"][:, :])
                ones1 = l1p.tile([1, P], BF16)
                nc.vector.memset(ones1[:], 1.0)
                for t in range(T):
                    pd1 = l1ps.tile([P, FM], F32, tag="pd1", name=f"pd1_{t}")
                    nc.tensor.matmul(out=pd1[:], lhsT=xT1_sb[:, t * P:(t + 1) * P],
                                     rhs=W1v_sb[:], start=True, stop=False)
                    nc.tensor.matmul(out=pd1[:], lhsT=ones1[:], rhs=hcrow_sb[:],
                                     start=False, stop=True)
                    nc.scalar.activation(out=hres[t][:], in_=pd1[:], func=Ident,
                                         scale=dinv2_sb[:, t:t + 1])
                    nc.sync.dma_start(out=h512[0][t * P:(t + 1) * P, :],
                                      in_=hres[t][:])

            # ---- merged phases: scatter(i) + dense(i+1), i = 1..4 ----
            import os as _os
            SBUFS = int(_os.environ.get("KBASS_BUFS", "7"))
            for i in (1, 2, 3, 4):
                relu = i in (2, 4)
                h_src = h512[i - 1]
                F_out = FM if i < 4 else F5
                W_d = d[f"W{i + 1}"]
                with tc.tile_pool(name=f"ph{i}", bufs=SBUFS) as sp, \
                     tc.tile_pool(name=f"ph{i}w", bufs=1) as wp, \
                     tc.tile_pool(name=f"ph{i}ps", bufs=3, space="PSUM") as pp, \
                     tc.tile_pool(name=f"ph{i}pt", bufs=2, space="PSUM") as pt, \
                     tc.tile_pool(name=f"ph{i}pd", bufs=2, space="PSUM") as pd:
                    # bias row of the NEXT phase's gather table
                    if i < 4:
                        nc.sync.dma_start(out=h512[i][NP:NP + 1, :],
                                          in_=brows_sb[i:i + 1, :])
                    else:
                        nc.sync.dma_start(out=h5t[NP:NP + 1, :],
                                          in_=brows56_sb[0:1, :])
                    W_sb = [wp.tile([P, F_out], BF16, tag=f"w{k}", name=f"w{i}_{k}")
                            for k in range(KM)]
                    for k in range(KM):
                        nc.sync.dma_start(out=W_sb[k][:], in_=W_d[k * P:(k + 1) * P, :])
                    for t in range(T):
                        g = gather(sp, h_src, t, FM, f"g{i}")
                        oh = load_oh(sp, t, f"oh{i}")
                        pa = pp.tile([P, FM], F32, tag="pa", name=f"pa{i}_{t}")
                        for c in range(C):
                            nc.tensor.matmul(
                                out=pa[:], lhsT=oh[:, c * P:(c + 1) * P],
                                rhs=g[:, c * FM:(c + 1) * FM],
                                start=(c == 0), stop=(c == C - 1))
                        # self-loop (h rows pre-scaled by dinv^2) fused into
                        # the PSUM->SBUF drain on DVE; bias came via the
                        # all-ones one-hot slot
                        node = sp.tile([P, FM], BF16, tag="node", name=f"nd{i}_{t}")
                        nc.vector.tensor_add(out=node[:], in0=pa[:],
                                             in1=hres[t][:])
                        ptr = pt.tile([P, FM], F32, tag="ptr", name=f"pt{i}_{t}")
                        for fo in range(KM):
                            nc.tensor.matmul(
                                out=ptr[:, fo * P:(fo + 1) * P],
                                lhsT=node[:, fo * P:(fo + 1) * P],
                                rhs=ident_b[:],
                                start=True, stop=True)
                        stage = sp.tile([P, FM], BF16, tag="stage", name=f"st{i}_{t}")
                        nc.scalar.activation(out=stage[:], in_=ptr[:],
                                             func=Relu if relu else Ident)
                        pdn = pd.tile([P, F_out], F32, tag="pdn", name=f"pd{i}_{t}")
                        for k in range(KM):
                            nc.tensor.matmul(out=pdn[:],
                                             lhsT=stage[:, k * P:(k + 1) * P],
                                             rhs=W_sb[k][:], start=(k == 0),
                                             stop=(k == KM - 1))
                        if i < 4:
                            nc.scalar.activation(out=hres[t][:], in_=pdn[:],
                                                 func=Ident,
                                                 scale=dinv2_sb[:, t:t + 1])
                            nc.sync.dma_start(out=h512[i][t * P:(t + 1) * P, :],
                                              in_=hres[t][:])
                        else:
                            nc.scalar.activation(out=hres5[t][:, 0:F5],
                                                 in_=pdn[:], func=Ident,
                                                 scale=dinv2_sb[:, t:t + 1])
                            nc.sync.dma_start(out=h5t[t * P:(t + 1) * P, :],
                                              in_=hres5[t][:])

            # ---- phase 5: x6 = (A_hat h5 + b5) * dinv2, 64-wide ----
            with tc.tile_pool(name="s5", bufs=SBUFS) as sp5, \
                 tc.tile_pool(name="s5ps", bufs=2, space="PSUM") as pp5, \
                 tc.tile_pool(name="s5pt", bufs=2, space="PSUM") as pt5:
                nc.sync.dma_start(out=x6t[NP:NP + 1, :], in_=brows56_sb[1:2, :])
                for t in range(T):
                    g5 = gather(sp5, h5t, t, P, "g5")
                    oh5 = load_oh(sp5, t, "oh5")
                    pg5 = pp5.tile([F5, P], F32, tag="pg5", name=f"pg5_{t}")
                    nc.tensor.matmul(out=pg5[:], lhsT=hres5[t][:, 0:F5],
                                     rhs=ident_b[:],
                                     start=True, stop=False,
                                     skip_group_check=True)
                    for c in range(C):
                        nc.tensor.matmul(out=pg5[:],
                                         lhsT=g5[:, c * P:c * P + F5],
                                         rhs=oh5[:, c * P:(c + 1) * P],
                                         start=False, stop=(c == C - 1),
                                         skip_group_check=True)
                    st6 = sp5.tile([F5, P], BF16, tag="st6", name=f"st6_{t}")
                    nc.scalar.activation(out=st6[:], in_=pg5[:], func=Ident)
                    pt6 = pt5.tile([P, F5], F32, tag="pt6", name=f"pt6_{t}")
                    nc.tensor.matmul(out=pt6[:], lhsT=st6[:],
                                     rhs=ident_b[0:F5, 0:F5],
                                     start=True, stop=True)
                    nc.scalar.activation(out=hres5[t][:, 0:F5], in_=pt6[:],
                                         func=Ident,
                                         scale=dinv2_sb[:, t:t + 1])
                    nc.sync.dma_start(out=x6t[t * P:(t + 1) * P, :],
                                      in_=hres5[t][:])

            # ---- phase 6: out = (A_hat x6) @ W6 + b6 ----
            with tc.tile_pool(name="s6", bufs=SBUFS) as sp6, \
                 tc.tile_pool(name="s6w", bufs=1) as wp6, \
                 tc.tile_pool(name="s6ps", bufs=2, space="PSUM") as pp6, \
                 tc.tile_pool(name="s6pd", bufs=2, space="PSUM") as po6:
                W6_sb = wp6.tile([F5, FO], BF16)
                nc.sync.dma_start(out=W6_sb[:], in_=d["W6"][:, :])
                for t in range(T):
                    g6 = gather(sp6, x6t, t, P, "g6")
                    oh6 = load_oh(sp6, t, "oh6")
                    pg6 = pp6.tile([F5, P], F32, tag="pg6", name=f"pg6_{t}")
                    nc.tensor.matmul(out=pg6[:], lhsT=hres5[t][:, 0:F5],
                                     rhs=ident_b[:],
                                     start=True, stop=False,
                                     skip_group_check=True)
                    for c in range(C):
                        nc.tensor.matmul(out=pg6[:],
                                         lhsT=g6[:, c * P:c * P + F5],
                                         rhs=oh6[:, c * P:(c + 1) * P],
                                         start=False, stop=(c == C - 1),
                                         skip_group_check=True)
                    st7 = sp6.tile([F5, P], BF16, tag="st7", name=f"st7_{t}")
                    nc.scalar.activation(out=st7[:], in_=pg6[:], func=Ident)
                    pout = po6.tile([P, FO], F32, tag="pout", name=f"po_{t}")
                    nc.tensor.matmul(out=pout[:], lhsT=st7[:], rhs=W6_sb[:],
                                     start=True, stop=True)
                    os_ = sp6.tile([P, FO], F32, tag="os", name=f"o_{t}")
                    nc.vector.tensor_add(out=os_[:], in0=pout[:], in1=b6r_sb[:])
                    nc.sync.dma_start(out=out_d[t * P:(t + 1) * P, :], in_=os_[:])

    nc.compile()
    return nc


def _prepare(batch_vertices, img_features, edge_indices,
             W1, b1, W2, b2, W3, b3, W4, b4, W5, b5, W6, b6):
    B, N, _ = batch_vertices.shape
    FM = W1.shape[1]
    F5 = W5.shape[1]
    FO = W6.shape[1]

    ei = np.asarray(edge_indices).astype(np.int64)
    g = _pack_graph(ei[0], ei[1], N)
    NP, T, C, SW, perm = g["NP"], g["T"], g["C"], g["SW"], g["perm"]

    hc = img_features.astype(np.float32) @ W1[3:].astype(np.float32)

    valid = perm >= 0
    vperm = np.zeros((B, NP, 3), np.float32)
    vperm[:, valid, :] = batch_vertices[:, perm[valid], :]

    b5row = np.zeros((1, P), np.float32)
    b5row[0, :F5] = b5
    common = {
        "W1v": np.ascontiguousarray(W1[:3]).astype(BF),
        "W2": np.ascontiguousarray(W2).astype(BF),
        "W3": np.ascontiguousarray(W3).astype(BF),
        "W4": np.ascontiguousarray(W4).astype(BF),
        "W5": np.ascontiguousarray(W5).astype(BF),
        "W6": np.ascontiguousarray(W6).astype(BF),
        "Brows": np.stack([b1, b2, b3, b4]).astype(BF),
        "Brows56": np.concatenate([b5row, np.zeros((1, P), np.float32)]).astype(BF),
        "b6rep": np.tile(b6.astype(np.float32), (P, 1)),
        "idx16": g["idx16"], "slotb": g["slot"], "normb": g["norm"],
        "dinv2": g["dinv2"],
    }
    in_maps = []
    for b in range(B):
        m = dict(common)
        m["xT1"] = np.ascontiguousarray(vperm[b].T).astype(BF)
        m["hcrow"] = hc[b].reshape(1, FM).astype(BF)
        in_maps.append(m)
    meta = dict(NP=NP, T=T, C=C, SW=SW, perm=perm, valid=valid, B=B, N=N,
                FM=FM, F5=F5, FO=FO)
    return in_maps, meta


_BUILD_CACHE = {}


def run(inputs, trace=False):
    in_maps, meta = _prepare(**inputs)
    key = (meta["NP"], meta["C"], meta["FM"], meta["F5"], meta["FO"])
    if key not in _BUILD_CACHE:
        t0 = time.time()
        _BUILD_CACHE[key] = _build_nc(meta["NP"], meta["T"], meta["C"],
                                      meta["SW"], meta["FM"], meta["F5"],
                                      meta["FO"])
        print(f"[kernel] built bass program in {time.time()-t0:.1f}s", file=sys.stderr)
    nc = _BUILD_CACHE[key]
    B = meta["B"]
    res = run_bass_kernel_spmd(nc, in_maps, core_ids=list(range(B)), trace=trace)
    perm, valid, N = meta["perm"], meta["valid"], meta["N"]
    out = np.empty((B, N, meta["FO"]), np.float32)
    for b in range(B):
        dev = res.results[b]["out"]
        out[b, perm[valid], :] = dev[valid, :]
    return out, res


def kernel(**inputs) -> np.ndarray:
    out, _ = run(inputs)
    return out
